# revision 21
# baseline (speedup 1.0000x reference)
"""Multi-head attention + residual + LayerNorm on 8 Trainium2 cores.

Sharding (per spec hint): core c = (batch b = c//4, head-group hg = c%4 of
4 heads).  Two SPMD launches:

Launch 1 (attention, per core):
  - Q/K/V projections as fp8e4m3 DoubleRow matmuls (2 k-tiles per pass,
    0.5 cycles/row).  The 1/8 score scale is folded into the Q weights;
    a +5.0625 score bias comes from a constant contraction row (2.25 in
    both Q and K operands), so PSUM scores arrive as s/8 + 5.0625.
  - scores^T per (head, k-chunk, q-block) as one DoubleRow matmul with
    d_k split 2x32 (dk-split layout built by an SBUF->SBUF DMA regroup
    after the projections).
  - softmax exp is split across TWO engines: ACT computes
    exp(in - 4.85577) -> fp8e4m3; DVE computes the same value (x1.2296,
    cancels in softmax) with an integer exp trick: u8 = trunc(max(
    in*11.5416, 0)) bit-cast as fp8e4m3.  Split tunable via SLOT_ENG.
  - ctx^T accumulated with fp8 DoubleRow matmuls over k-chunk pairs; a
    ones-column in the V operand makes row 64 the softmax denominator.
    ctx (65 rows: 64 dims + denom) leaves unnormalized in bf16.

Launch 2 (normalize + out-projection + residual + LayerNorm, 512 rows):
  reciprocal of the denominators, e16-matmul broadcast, one psum x sbuf
  multiply -> fp8 ctx, fp8 DoubleRow out-projection, residual add,
  bn_stats LayerNorm; gamma/beta applied on Pool to keep DVE short.
"""

from contextlib import ExitStack

import numpy as np
import ml_dtypes

import concourse.bass as bass
import concourse.bacc as bacc
import concourse.tile as tile
from concourse import mybir
from concourse.bass_utils import run_bass_kernel_spmd

BF16 = mybir.dt.bfloat16
F32 = mybir.dt.float32
FP8 = mybir.dt.float8e4
FP8E5 = mybir.dt.float8e5
U8 = mybir.dt.uint8
NPBF16 = ml_dtypes.bfloat16
NPFP8 = ml_dtypes.float8_e4m3
DR = mybir.MatmulPerfMode.DoubleRow

B, S, D = 2, 2048, 1024
H = 16
DK = 64
N_CORES = 8
H_LOC = 4           # heads per core
NCH = S // 128      # 16 k-chunks
NIC = D // 128      # 8 contraction chunks (4 DoubleRow pairs)
NQB = S // 512      # 4 q-blocks
EPS = 1e-5

# exp bias trick: scores arrive as s/8 + CBIAS (CBIAS = 2.25*2.25 via a
# constant contraction row).  DVE: u8 = trunc(max(in*EXPA, 0)) is the
# fp8e4m3 bit pattern of 1.2296*exp(s/8).  ACT matches via exp(in+ABIAS).
CBIAS = 8.0  # 2.0 * 4.0 const row; scores/8 in [-7.5, 9.5] all fit e5m2
EXPA = 5.7707801635558535  # 4*log2(e) for e5m2 bits
PSC = 1.0 / 2048.0   # psum scores carry 256x (16x-scaled Q,K); target score/8
ABIAS = -10.445207698461314  # -15*ln2 + ln(0.95313); matches DVE e5m2 trick

# exp slot -> engine map per (h, qb) block: 8 slots of 2 k-chunks.
# 'A' = ACT, 'D' = DVE (vector); two patterns alternate by block parity.
SLOT_PATS = ("ADADADAD", "AADADADA")

E16_HOST = np.zeros((H, NIC * 128), NPBF16)
for _ic in range(NIC):
    for _j in range(2):
        E16_HOST[2 * _ic + _j, 128 * _ic + 64 * _j : 128 * _ic + 64 * _j + 64] = 1.0

_cache = {}


def build_kernel1():
    nc = bacc.Bacc("TRN2", target_bir_lowering=False, debug=False)

    xq = nc.dram_tensor("xq", [D, S], FP8, kind="ExternalInput")
    xk = nc.dram_tensor("xk", [D, S], FP8, kind="ExternalInput")
    xv = nc.dram_tensor("xv", [D, S], FP8, kind="ExternalInput")
    wq = nc.dram_tensor("wq", [D, 256], FP8, kind="ExternalInput")
    wk = nc.dram_tensor("wk", [D, 256], FP8, kind="ExternalInput")
    wv = nc.dram_tensor("wv", [D, 256], FP8, kind="ExternalInput")
    # 65 rows per head: 64 ctx dims + softmax denominator (unnormalized)
    ctxu = nc.dram_tensor("ctxu", [65 * H_LOC, S], BF16, kind="ExternalOutput")

    with tile.TileContext(nc) as tc:
        with (
            tc.tile_pool(name="wp", bufs=1) as wp,
            tc.tile_pool(name="xp", bufs=1) as xp,
            tc.tile_pool(name="qks", bufs=1) as qksp,
            tc.tile_pool(name="va", bufs=1) as vap,
            tc.tile_pool(name="sx", bufs=3) as sxp,
            tc.tile_pool(name="cu", bufs=2) as cup,
            tc.tile_pool(name="sm", bufs=1) as smp,
            tc.tile_pool(name="ps", bufs=3, space="PSUM") as psp,      # score slots [128,1024] x3
            tc.tile_pool(name="pc", bufs=2, space="PSUM") as pcp,      # ctx [96,512] x2
        ):
            w_q = wp.tile([128, NIC, 256], FP8)
            w_k = wp.tile([128, NIC, 256], FP8)
            w_v = wp.tile([128, NIC, 256], FP8)
            x_q = xp.tile([128, NQB, NIC, 512], FP8)
            x_k = xp.tile([128, NQB, NIC, 512], FP8)
            x_v = xp.tile([128, NQB, NIC, 512], FP8)
            # dk-split Q/K: [33, h, pair, S]; row 32 pair0 = 2.25 (bias row)
            qs = qksp.tile([33, H_LOC, NQB, 2, 512], FP8)
            ks = qksp.tile([33, H_LOC, NQB, 2, 512], FP8)
            # staging for projection output (natural head-dim layout)
            qstg = qksp.tile([128, 2, 2, S], FP8)  # [dims, (q|k), hp, S]
            vaug = vap.tile([128, NCH, H_LOC, 96], FP8)
            abias = smp.tile([128, 1], F32)

            nc.vector.memset(abias[:], ABIAS)
            nc.gpsimd.memset(vaug[:, :, :, 64:96], 16.0)  # V is 16x-scaled; denom follows
            nc.gpsimd.memset(qs[32:33, :, :, 0, :], 128.0)
            nc.gpsimd.memset(qs[32:33, :, :, 1, :], 0.0)
            nc.gpsimd.memset(ks[32:33, :, :, 0, :], 128.0)  # 128*128/2048 = CBIAS 8
            nc.gpsimd.memset(ks[32:33, :, :, 1, :], 0.0)

            # ---- input DMA: one multi-dim DMA per tensor ----
            def load_w(tile, dram):
                a = dram.ap()
                nc.sync.dma_start(out=tile[:], in_=bass.AP(
                    tensor=a.tensor, offset=0,
                    ap=[[256, 128], [128 * 256, NIC], [1, 256]]))

            def load_x(tile, dram, split=False):
                a = dram.ap()
                if split:
                    for qb in range(NQB):
                        nc.sync.dma_start(out=tile[:, qb, :, :], in_=bass.AP(
                            tensor=a.tensor, offset=512 * qb,
                            ap=[[S, 128], [128 * S, NIC], [1, 512]]))
                else:
                    nc.sync.dma_start(out=tile[:], in_=bass.AP(
                        tensor=a.tensor, offset=0,
                        ap=[[S, 128], [512, NQB], [128 * S, NIC], [1, 512]]))

            load_w(w_q, wq)
            load_w(w_k, wk)
            load_x(x_q, xq, split=True)
            load_x(x_k, xk, split=True)
            load_w(w_v, wv)
            load_x(x_v, xv)

            # ---- Q/K projections (fp8 DoubleRow), hp-granular ----
            def qk_proj(t, hp):
                xt = (x_q, x_k)[t]
                wt = (w_q, w_k)[t]
                for qb in range(NQB):
                    pt = psp.tile([128, 1024], F32, tag="slot")
                    for icp in range(NIC // 2):
                        nc.tensor.matmul(
                            pt[:, 0:512],
                            wt[:, 2 * icp : 2 * icp + 2, 128 * hp : 128 * hp + 128],
                            xt[:, qb, 2 * icp : 2 * icp + 2, :],
                            start=(icp == 0),
                            stop=(icp == NIC // 2 - 1),
                            perf_mode=DR,
                        )
                    nc.scalar.copy(
                        out=qstg[:, t, hp, 512 * qb : 512 * qb + 512], in_=pt[:, 0:512]
                    )

            def qk_regroup(t, hp):
                # [128, S] staging -> dk-split [32, h, j, S] (+bias row set above)
                dst = (qs, ks)[t]
                for blk in range(4):
                    h, j = 2 * hp + blk // 2, blk % 2
                    nc.sync.dma_start(
                        out=dst[0:32, h, :, j, :],
                        in_=qstg[32 * blk : 32 * blk + 32, t, hp, :],
                    )

            def v_proj(c):
                pvt = psp.tile([128, 1024], F32, tag="slot")
                for icp in range(NIC // 2):
                    nc.tensor.matmul(
                        pvt[:, 0:256],
                        x_v[:, c // 4, 2 * icp : 2 * icp + 2, 128 * (c % 4) : 128 * (c % 4) + 128],
                        w_v[:, 2 * icp : 2 * icp + 2, :],
                        start=(icp == 0),
                        stop=(icp == NIC // 2 - 1),
                        perf_mode=DR,
                    )
                nc.vector.tensor_copy(out=vaug[:, c, :, 0:64], in_=pvt[:, 0:256])

            for hp in range(2):
                qk_proj(0, hp)
                qk_proj(1, hp)
                qk_regroup(0, hp)
                qk_regroup(1, hp)

            # ---- main attention loop ----
            blocks = [(h, qb) for h in range(H_LOC) for qb in range(NQB)]
            exps = {}    # (h, qb) -> expS tile
            pctxs = {}   # (h, qb) -> ctx psum

            def scores_block(h, qb, vblk=None):
                ex = sxp.tile([128, NCH, 512], FP8E5, tag="expS")
                exps[(h, qb)] = ex
                for sl in range(8):
                    if vblk is not None:
                        v_proj(8 * vblk + sl)
                    slot = psp.tile([128, 1024], F32, tag="slot")
                    for k in range(2):
                        c = 2 * sl + k
                        nc.tensor.matmul(
                            slot[:, 512 * k : 512 * k + 512],
                            ks[0:33, h, c // 4, :, 128 * (c % 4) : 128 * (c % 4) + 128],
                            qs[0:33, h, qb, :, :],
                            start=True,
                            stop=True,
                            perf_mode=DR,
                        )
                    eng = SLOT_PATS[(H_LOC * h + qb) % 2][sl]
                    dst = ex[:, 2 * sl : 2 * sl + 2, :]
                    if eng == "A":
                        nc.scalar.activation(
                            out=dst,
                            in_=slot[:],
                            func=mybir.ActivationFunctionType.Exp,
                            bias=abias[:],
                            scale=PSC,
                        )
                    else:
                        nc.vector.tensor_scalar(
                            out=dst.bitcast(U8),
                            in0=slot[:],
                            scalar1=EXPA * PSC,
                            scalar2=0.0,
                            op0=mybir.AluOpType.mult,
                            op1=mybir.AluOpType.max,
                        )

            def ctx_block(h, qb):
                ex = exps.pop((h, qb))
                pctx = pcp.tile([96, 512], F32, tag="ctx")
                pctxs[(h, qb)] = pctx
                for i in range(8):
                    nc.tensor.matmul(
                        pctx[:],
                        vaug[:, 2 * i : 2 * i + 2, h, :],
                        ex[:, 2 * i : 2 * i + 2, :],
                        start=(i == 0),
                        stop=(i == 7),
                        perf_mode=DR,
                    )

            def finish_block(h, qb):
                pctx = pctxs.pop((h, qb))
                cu = cup.tile([65, 512], BF16, tag="cu")
                if (H_LOC * h + qb) % 2 == 0:
                    nc.scalar.copy(out=cu[:], in_=pctx[0:65, :])
                else:
                    nc.vector.tensor_copy(out=cu[:], in_=pctx[0:65, :])
                nc.sync.dma_start(
                    out=ctxu[65 * h : 65 * h + 65, 512 * qb : 512 * qb + 512],
                    in_=cu[:],
                )

            prev = None
            for bi, blk in enumerate(blocks):
                scores_block(*blk, vblk=bi if bi < 2 else None)
                if prev is not None:
                    ctx_block(*prev)
                    finish_block(*prev)
                prev = blk
            ctx_block(*prev)
            finish_block(*prev)

    nc.compile()
    return nc


def build_kernel2():
    nc = bacc.Bacc("TRN2", target_bir_lowering=False, debug=False)

    R = 512  # rows per core
    ctxTg = nc.dram_tensor("ctxTg", [D, R], BF16, kind="ExternalInput")
    dnm = nc.dram_tensor("dnm", [H, R], BF16, kind="ExternalInput")
    e16d = nc.dram_tensor("e16", [H, NIC * 128], BF16, kind="ExternalInput")
    woT = nc.dram_tensor("woT", [D, D], BF16, kind="ExternalInput")
    xres = nc.dram_tensor("xres", [R, D], F32, kind="ExternalInput")
    gamma = nc.dram_tensor("gamma", [1, D], F32, kind="ExternalInput")
    beta = nc.dram_tensor("beta", [1, D], F32, kind="ExternalInput")
    out = nc.dram_tensor("out", [R, D], F32, kind="ExternalOutput")

    with tile.TileContext(nc) as tc:
        with (
            tc.tile_pool(name="wo", bufs=1) as wop,
            tc.tile_pool(name="cx", bufs=1) as cxp,
            tc.tile_pool(name="sm", bufs=1) as smp,
            tc.tile_pool(name="wk", bufs=3) as wkp,
            tc.tile_pool(name="ps", bufs=1, space="PSUM") as psp,
            tc.tile_pool(name="pb", bufs=2, space="PSUM") as pbp,
        ):
            wo_t = wop.tile([128, NIC, D], BF16)
            ctx_t = cxp.tile([128, NIC, R], BF16)
            dnm_t = smp.tile([H, R], BF16)
            nc.sync.dma_start(out=dnm_t[:], in_=dnm[:])
            ca = ctxTg.ap()
            nc.sync.dma_start(out=ctx_t[:], in_=bass.AP(
                tensor=ca.tensor, offset=0,
                ap=[[R, 128], [128 * R, NIC], [1, R]]))
            wa = woT.ap()
            nc.sync.dma_start(out=wo_t[:], in_=bass.AP(
                tensor=wa.tensor, offset=0,
                ap=[[D, 128], [128 * D, NIC], [1, D]]))
            rec_t = smp.tile([H, R], BF16)
            with nc.allow_low_precision(reason="softmax denom recip in bf16; residual dominates"):
                nc.vector.reciprocal(out=rec_t[:], in_=dnm_t[:])

            e16 = smp.tile([H, NIC, 128], BF16)
            nc.sync.dma_start(out=e16[:].rearrange("h a b -> h (a b)"), in_=e16d[:])

            gb = smp.tile([128, D], F32)
            bb = smp.tile([128, D], F32)
            g_ap = gamma.ap()
            b_ap = beta.ap()
            nc.sync.dma_start(
                out=gb[:], in_=bass.AP(tensor=g_ap.tensor, offset=g_ap.offset,
                                       ap=[[0, 128], [1, D]])
            )
            nc.sync.dma_start(
                out=bb[:], in_=bass.AP(tensor=b_ap.tensor, offset=b_ap.offset,
                                       ap=[[0, 128], [1, D]])
            )
            eps_t = smp.tile([128, 1], F32)
            nc.vector.memset(eps_t[:], EPS)

            # normalize ctx^T by per-(head, row) denominators -> bf16 tiles,
            # pipelined into the out-projection accumulation
            ctxn = cxp.tile([128, NIC, R], BF16)
            pos = {}
            for sc in range(4):
                po_t = psp.tile([128, D], F32, tag=f"po{sc % 2}", name=f"po_{sc}")
                pos[sc] = po_t
            for ic in range(NIC):
                pb = pbp.tile([128, R], F32, tag="pb")
                nc.tensor.matmul(pb[:], e16[:, ic, :], rec_t[:], start=True, stop=True)
                nc.vector.tensor_tensor(
                    out=ctxn[:, ic, :], in0=pb[:], in1=ctx_t[:, ic, :],
                    op=mybir.AluOpType.mult,
                )
                for sc in range(2):
                    for j in range(2):
                        nc.tensor.matmul(
                            pos[sc][:, 512 * j : 512 * j + 512],
                            ctxn[:, ic, 128 * sc : 128 * sc + 128],
                            wo_t[:, ic, 512 * j : 512 * j + 512],
                            start=(ic == 0),
                            stop=(ic == NIC - 1),
                        )

            for sc in range(4):
                po = pos[sc]
                if sc >= 2:
                    for j in range(2):
                        for ic in range(NIC):
                            nc.tensor.matmul(
                                po[:, 512 * j : 512 * j + 512],
                                ctxn[:, ic, 128 * sc : 128 * sc + 128],
                                wo_t[:, ic, 512 * j : 512 * j + 512],
                                start=(ic == 0),
                                stop=(ic == NIC - 1),
                            )
                xq_sb = wkp.tile([128, D], F32, tag="xq")
                nc.sync.dma_start(out=xq_sb[:], in_=xres[128 * sc : 128 * sc + 128, :])
                x_sb = wkp.tile([128, D], F32, tag="x")
                nc.vector.tensor_add(out=x_sb[:], in0=po[:], in1=xq_sb[:])

                stats = wkp.tile([128, 2, 6], F32, tag="bn")
                for g in range(2):
                    nc.vector.bn_stats(out=stats[:, g, :], in_=x_sb[:, 512 * g : 512 * g + 512])
                mv = wkp.tile([128, 2], F32, tag="mv")
                nc.vector.bn_aggr(out=mv[:], in_=stats[:])
                std = wkp.tile([128, 1], F32, tag="std")
                nc.scalar.activation(
                    out=std[:], in_=mv[:, 1:2],
                    func=mybir.ActivationFunctionType.Sqrt,
                    bias=eps_t[:], scale=1.0,
                )
                rstd = wkp.tile([128, 1], F32, tag="rstd")
                nc.vector.reciprocal(out=rstd[:], in_=std[:])
                xn = wkp.tile([128, D], F32, tag="xn")
                nc.vector.tensor_scalar(
                    out=xn[:], in0=x_sb[:],
                    scalar1=mv[:, 0:1], scalar2=rstd[:],
                    op0=mybir.AluOpType.subtract, op1=mybir.AluOpType.mult,
                )
                xg = wkp.tile([128, D], F32, tag="xg")
                nc.vector.tensor_mul(out=xg[:], in0=xn[:], in1=gb[:])
                xb = wkp.tile([128, D], F32, tag="xb")
                nc.gpsimd.tensor_add(out=xb[:], in0=xg[:], in1=bb[:])
                nc.sync.dma_start(out=out[128 * sc : 128 * sc + 128, :], in_=xb[:])

    nc.compile()
    return nc


def _get(name):
    if name not in _cache:
        _cache[name] = build_kernel1() if name == "k1" else build_kernel2()
    return _cache[name]


def kernel(query, key, value, w_q, w_k, w_v, w_o, ln_gamma, ln_beta):
    query = np.asarray(query, np.float32)
    key = np.asarray(key, np.float32)
    value = np.asarray(value, np.float32)
    w_q = np.asarray(w_q, np.float32)
    w_k = np.asarray(w_k, np.float32)
    w_v = np.asarray(w_v, np.float32)
    w_o = np.asarray(w_o, np.float32)
    ln_gamma = np.asarray(ln_gamma, np.float32)
    ln_beta = np.asarray(ln_beta, np.float32)

    nc1 = _get("k1")
    nc2 = _get("k2")

    xqT = [np.ascontiguousarray(query[b].T).astype(NPFP8) for b in range(B)]
    xkT = [np.ascontiguousarray(key[b].T).astype(NPFP8) for b in range(B)]
    xvT = [np.ascontiguousarray(value[b].T).astype(NPFP8) for b in range(B)]
    # 16x scale keeps Xavier weights in e4m3 normal range (exact 2^k)
    wqT = np.ascontiguousarray(w_q.T * 16.0).astype(NPFP8)
    wkT = np.ascontiguousarray(w_k.T * 16.0).astype(NPFP8)
    wvT = np.ascontiguousarray(w_v.T * 16.0).astype(NPFP8)

    in_maps1 = []
    for c in range(N_CORES):
        b, hg = c // 4, c % 4
        in_maps1.append({
            "xq": xqT[b], "xk": xkT[b], "xv": xvT[b],
            "wq": np.ascontiguousarray(wqT[:, 256 * hg : 256 * hg + 256]),
            "wk": np.ascontiguousarray(wkT[:, 256 * hg : 256 * hg + 256]),
            "wv": np.ascontiguousarray(wvT[:, 256 * hg : 256 * hg + 256]),
        })
    res1 = run_bass_kernel_spmd(nc1, in_maps1, core_ids=list(range(N_CORES)))

    ctx_full = np.empty((D, B * S), NPBF16)
    dnm_full = np.empty((H, B * S), NPBF16)
    for c in range(N_CORES):
        b, hg = c // 4, c % 4
        cu = res1.results[c]["ctxu"]  # [65*4, 2048]
        for h in range(H_LOC):
            ctx_full[256 * hg + 64 * h : 256 * hg + 64 * h + 64, S * b : S * b + S] = \
                cu[65 * h : 65 * h + 64]
            dnm_full[4 * hg + h, S * b : S * b + S] = cu[65 * h + 64]

    woT = np.ascontiguousarray(w_o.T).astype(NPBF16)
    q_flat = query.reshape(B * S, D)
    g2 = ln_gamma.reshape(1, D)
    b2 = ln_beta.reshape(1, D)

    in_maps2 = []
    for c in range(N_CORES):
        r0 = 512 * c
        in_maps2.append({
            "ctxTg": np.ascontiguousarray(ctx_full[:, r0 : r0 + 512]),
            "dnm": np.ascontiguousarray(dnm_full[:, r0 : r0 + 512]),
            "e16": E16_HOST,
            "woT": woT,
            "xres": np.ascontiguousarray(q_flat[r0 : r0 + 512, :]),
            "gamma": g2, "beta": b2,
        })
    res2 = run_bass_kernel_spmd(nc2, in_maps2, core_ids=list(range(N_CORES)))

    out = np.concatenate([res2.results[c]["out"] for c in range(N_CORES)], axis=0)
    return out.reshape(B, S, D)


# revision 22
# speedup vs baseline: 1.0401x; 1.0401x over previous
"""Multi-head attention + residual + LayerNorm on 8 Trainium2 cores.

Sharding (per spec hint): core c = (batch b = c//4, head-group hg = c%4 of
4 heads).  Two SPMD launches:

Launch 1 (attention, per core):
  - Q/K/V projections as fp8e4m3 DoubleRow matmuls (2 k-tiles per pass,
    0.5 cycles/row).  The 1/8 score scale is folded into the Q weights;
    a +5.0625 score bias comes from a constant contraction row (2.25 in
    both Q and K operands), so PSUM scores arrive as s/8 + 5.0625.
  - scores^T per (head, k-chunk, q-block) as one DoubleRow matmul with
    d_k split 2x32 (dk-split layout built by an SBUF->SBUF DMA regroup
    after the projections).
  - softmax exp is split across TWO engines: ACT computes
    exp(in - 4.85577) -> fp8e4m3; DVE computes the same value (x1.2296,
    cancels in softmax) with an integer exp trick: u8 = trunc(max(
    in*11.5416, 0)) bit-cast as fp8e4m3.  Split tunable via SLOT_ENG.
  - ctx^T accumulated with fp8 DoubleRow matmuls over k-chunk pairs; a
    ones-column in the V operand makes row 64 the softmax denominator.
    ctx (65 rows: 64 dims + denom) leaves unnormalized in bf16.

Launch 2 (normalize + out-projection + residual + LayerNorm, 512 rows):
  reciprocal of the denominators, e16-matmul broadcast, one psum x sbuf
  multiply -> fp8 ctx, fp8 DoubleRow out-projection, residual add,
  bn_stats LayerNorm; gamma/beta applied on Pool to keep DVE short.
"""

from contextlib import ExitStack

import numpy as np
import ml_dtypes

import concourse.bass as bass
import concourse.bacc as bacc
import concourse.tile as tile
from concourse import mybir
from concourse.bass_utils import run_bass_kernel_spmd

BF16 = mybir.dt.bfloat16
F32 = mybir.dt.float32
FP8 = mybir.dt.float8e4
FP8E5 = mybir.dt.float8e5
U8 = mybir.dt.uint8
NPBF16 = ml_dtypes.bfloat16
NPFP8 = ml_dtypes.float8_e4m3
DR = mybir.MatmulPerfMode.DoubleRow

B, S, D = 2, 2048, 1024
H = 16
DK = 64
N_CORES = 8
H_LOC = 4           # heads per core
NCH = S // 128      # 16 k-chunks
NIC = D // 128      # 8 contraction chunks (4 DoubleRow pairs)
NQB = S // 512      # 4 q-blocks
EPS = 1e-5

# exp bias trick: scores arrive as s/8 + CBIAS (CBIAS = 2.25*2.25 via a
# constant contraction row).  DVE: u8 = trunc(max(in*EXPA, 0)) is the
# fp8e4m3 bit pattern of 1.2296*exp(s/8).  ACT matches via exp(in+ABIAS).
CBIAS = 8.0  # 2.0 * 4.0 const row; scores/8 in [-7.5, 9.5] all fit e5m2
EXPA = 5.7707801635558535  # 4*log2(e) for e5m2 bits
PSC = 1.0 / 2048.0   # psum scores carry 256x (16x-scaled Q,K); target score/8
ABIAS = -10.445207698461314  # -15*ln2 + ln(0.95313); matches DVE e5m2 trick

# exp slot -> engine map per (h, qb) block: 8 slots of 2 k-chunks.
# 'A' = ACT, 'D' = DVE (vector); two patterns alternate by block parity.
SLOT_PATS = ("ADADADAD", "AADADADA")

E16_HOST = np.zeros((H, NIC * 128), NPBF16)
for _ic in range(NIC):
    for _j in range(2):
        E16_HOST[2 * _ic + _j, 128 * _ic + 64 * _j : 128 * _ic + 64 * _j + 64] = 1.0

_cache = {}


def build_kernel1():
    nc = bacc.Bacc("TRN2", target_bir_lowering=False, debug=False)

    xq = nc.dram_tensor("xq", [D, S], FP8, kind="ExternalInput")
    xk = nc.dram_tensor("xk", [D, S], FP8, kind="ExternalInput")
    xv = nc.dram_tensor("xv", [D, S], FP8, kind="ExternalInput")
    wq = nc.dram_tensor("wq", [D, 256], FP8, kind="ExternalInput")
    wk = nc.dram_tensor("wk", [D, 256], FP8, kind="ExternalInput")
    wv = nc.dram_tensor("wv", [D, 256], FP8, kind="ExternalInput")
    # 65 rows per head: 64 ctx dims + softmax denominator (unnormalized)
    ctxu = nc.dram_tensor("ctxu", [65 * H_LOC, S], BF16, kind="ExternalOutput")

    with tile.TileContext(nc) as tc:
        with (
            tc.tile_pool(name="wp", bufs=1) as wp,
            tc.tile_pool(name="xp", bufs=1) as xp,
            tc.tile_pool(name="qks", bufs=1) as qksp,
            tc.tile_pool(name="va", bufs=1) as vap,
            tc.tile_pool(name="sx", bufs=3) as sxp,
            tc.tile_pool(name="cu", bufs=2) as cup,
            tc.tile_pool(name="sm", bufs=1) as smp,
            tc.tile_pool(name="ps", bufs=3, space="PSUM") as psp,      # score slots [128,1024] x3
            tc.tile_pool(name="pc", bufs=2, space="PSUM") as pcp,      # ctx [96,512] x2
        ):
            w_q = wp.tile([128, NIC, 256], FP8)
            w_k = wp.tile([128, NIC, 256], FP8)
            w_v = wp.tile([128, NIC, 256], FP8)
            x_q = xp.tile([128, NQB, NIC, 512], FP8)
            x_k = xp.tile([128, NQB, NIC, 512], FP8)
            x_v = xp.tile([128, NQB, NIC, 512], FP8)
            # dk-split Q/K: [33, h, pair, S]; row 32 pair0 = 2.25 (bias row)
            qs = qksp.tile([33, H_LOC, NQB, 2, 512], FP8)
            ks = qksp.tile([33, H_LOC, NQB, 2, 512], FP8)
            # staging for projection output (natural head-dim layout)
            qstg = qksp.tile([128, 2, 2, S], FP8)  # [dims, (q|k), hp, S]
            vaug = vap.tile([128, NCH, H_LOC, 96], FP8)
            abias = smp.tile([128, 1], F32)

            nc.vector.memset(abias[:], ABIAS)
            nc.gpsimd.memset(vaug[:, :, :, 64:96], 16.0)  # V is 16x-scaled; denom follows
            nc.gpsimd.memset(qs[32:33, :, :, 0, :], 128.0)
            nc.gpsimd.memset(qs[32:33, :, :, 1, :], 0.0)
            nc.gpsimd.memset(ks[32:33, :, :, 0, :], 128.0)  # 128*128/2048 = CBIAS 8
            nc.gpsimd.memset(ks[32:33, :, :, 1, :], 0.0)

            # ---- input DMA: one multi-dim DMA per tensor ----
            def load_w(tile, dram):
                a = dram.ap()
                nc.sync.dma_start(out=tile[:], in_=bass.AP(
                    tensor=a.tensor, offset=0,
                    ap=[[256, 128], [128 * 256, NIC], [1, 256]]))

            def load_x(tile, dram, split=False):
                a = dram.ap()
                if split:
                    for qb in range(NQB):
                        nc.sync.dma_start(out=tile[:, qb, :, :], in_=bass.AP(
                            tensor=a.tensor, offset=512 * qb,
                            ap=[[S, 128], [128 * S, NIC], [1, 512]]))
                else:
                    nc.sync.dma_start(out=tile[:], in_=bass.AP(
                        tensor=a.tensor, offset=0,
                        ap=[[S, 128], [512, NQB], [128 * S, NIC], [1, 512]]))

            load_w(w_q, wq)
            load_w(w_k, wk)
            load_x(x_q, xq, split=True)
            load_x(x_k, xk, split=True)
            load_w(w_v, wv)
            load_x(x_v, xv)

            # ---- Q/K projections (fp8 DoubleRow), hp-granular ----
            def qk_proj(t, hp):
                xt = (x_q, x_k)[t]
                wt = (w_q, w_k)[t]
                for qb in range(NQB):
                    pt = psp.tile([128, 1024], F32, tag="slot")
                    for icp in range(NIC // 2):
                        nc.tensor.matmul(
                            pt[:, 0:512],
                            wt[:, 2 * icp : 2 * icp + 2, 128 * hp : 128 * hp + 128],
                            xt[:, qb, 2 * icp : 2 * icp + 2, :],
                            start=(icp == 0),
                            stop=(icp == NIC // 2 - 1),
                            perf_mode=DR,
                        )
                    nc.scalar.copy(
                        out=qstg[:, t, hp, 512 * qb : 512 * qb + 512], in_=pt[:, 0:512]
                    )

            def qk_regroup(t, hp):
                # [128, S] staging -> dk-split [32, h, j, S] (+bias row set above)
                dst = (qs, ks)[t]
                for blk in range(4):
                    h, j = 2 * hp + blk // 2, blk % 2
                    nc.sync.dma_start(
                        out=dst[0:32, h, :, j, :],
                        in_=qstg[32 * blk : 32 * blk + 32, t, hp, :],
                    )

            def v_proj(c):
                pvt = psp.tile([128, 1024], F32, tag="slot")
                for icp in range(NIC // 2):
                    nc.tensor.matmul(
                        pvt[:, 0:256],
                        x_v[:, c // 4, 2 * icp : 2 * icp + 2, 128 * (c % 4) : 128 * (c % 4) + 128],
                        w_v[:, 2 * icp : 2 * icp + 2, :],
                        start=(icp == 0),
                        stop=(icp == NIC // 2 - 1),
                        perf_mode=DR,
                    )
                nc.vector.tensor_copy(out=vaug[:, c, :, 0:64], in_=pvt[:, 0:256])

            for hp in range(2):
                qk_proj(0, hp)
                qk_proj(1, hp)
                qk_regroup(0, hp)
                qk_regroup(1, hp)

            # ---- main attention loop ----
            blocks = [(h, qb) for h in range(H_LOC) for qb in range(NQB)]
            exps = {}    # (h, qb) -> expS tile
            pctxs = {}   # (h, qb) -> ctx psum

            def scores_block(h, qb, vblk=None):
                ex = sxp.tile([128, NCH, 512], FP8E5, tag="expS")
                exps[(h, qb)] = ex
                for sl in range(8):
                    if vblk is not None:
                        v_proj(8 * vblk + sl)
                    slot = psp.tile([128, 1024], F32, tag="slot")
                    for k in range(2):
                        c = 2 * sl + k
                        nc.tensor.matmul(
                            slot[:, 512 * k : 512 * k + 512],
                            ks[0:33, h, c // 4, :, 128 * (c % 4) : 128 * (c % 4) + 128],
                            qs[0:33, h, qb, :, :],
                            start=True,
                            stop=True,
                            perf_mode=DR,
                        )
                    eng = SLOT_PATS[(H_LOC * h + qb) % 2][sl]
                    dst = ex[:, 2 * sl : 2 * sl + 2, :]
                    if eng == "A":
                        nc.scalar.activation(
                            out=dst,
                            in_=slot[:],
                            func=mybir.ActivationFunctionType.Exp,
                            bias=abias[:],
                            scale=PSC,
                        )
                    else:
                        nc.vector.tensor_scalar(
                            out=dst.bitcast(U8),
                            in0=slot[:],
                            scalar1=EXPA * PSC,
                            scalar2=0.0,
                            op0=mybir.AluOpType.mult,
                            op1=mybir.AluOpType.max,
                        )

            def ctx_block(h, qb):
                ex = exps.pop((h, qb))
                pctx = pcp.tile([96, 512], F32, tag="ctx")
                pctxs[(h, qb)] = pctx
                for i in range(8):
                    nc.tensor.matmul(
                        pctx[:],
                        vaug[:, 2 * i : 2 * i + 2, h, :],
                        ex[:, 2 * i : 2 * i + 2, :],
                        start=(i == 0),
                        stop=(i == 7),
                        perf_mode=DR,
                    )

            def finish_block(h, qb):
                pctx = pctxs.pop((h, qb))
                cu = cup.tile([65, 512], BF16, tag="cu")
                if (H_LOC * h + qb) % 2 == 0:
                    nc.scalar.copy(out=cu[:], in_=pctx[0:65, :])
                else:
                    nc.vector.tensor_copy(out=cu[:], in_=pctx[0:65, :])
                nc.sync.dma_start(
                    out=ctxu[65 * h : 65 * h + 65, 512 * qb : 512 * qb + 512],
                    in_=cu[:],
                )

            prev = None
            for bi, blk in enumerate(blocks):
                scores_block(*blk, vblk=bi if bi < 2 else None)
                if prev is not None:
                    ctx_block(*prev)
                    finish_block(*prev)
                prev = blk
            ctx_block(*prev)
            finish_block(*prev)

    nc.compile()
    return nc


def build_kernel2():
    nc = bacc.Bacc("TRN2", target_bir_lowering=False, debug=False)

    R = 512  # rows per core
    ctxTg = nc.dram_tensor("ctxTg", [D, R], BF16, kind="ExternalInput")
    dnm = nc.dram_tensor("dnm", [H, R], BF16, kind="ExternalInput")
    e16d = nc.dram_tensor("e16", [H, NIC * 128], BF16, kind="ExternalInput")
    woT = nc.dram_tensor("woT", [D, D], BF16, kind="ExternalInput")
    xres = nc.dram_tensor("xres", [R, D], F32, kind="ExternalInput")
    gamma = nc.dram_tensor("gamma", [1, D], F32, kind="ExternalInput")
    beta = nc.dram_tensor("beta", [1, D], F32, kind="ExternalInput")
    out = nc.dram_tensor("out", [R, D], F32, kind="ExternalOutput")

    with tile.TileContext(nc) as tc:
        with (
            tc.tile_pool(name="wo", bufs=1) as wop,
            tc.tile_pool(name="cx", bufs=1) as cxp,
            tc.tile_pool(name="sm", bufs=1) as smp,
            tc.tile_pool(name="wk", bufs=3) as wkp,
            tc.tile_pool(name="ps", bufs=1, space="PSUM") as psp,
            tc.tile_pool(name="pb", bufs=2, space="PSUM") as pbp,
        ):
            wo_t = wop.tile([128, NIC, D], BF16)
            ctx_t = cxp.tile([128, NIC, R], BF16)
            dnm_t = smp.tile([H, R], BF16)
            nc.sync.dma_start(out=dnm_t[:], in_=dnm[:])
            e16 = smp.tile([H, NIC, 128], BF16)
            nc.sync.dma_start(out=e16[:].rearrange("h a b -> h (a b)"), in_=e16d[:])
            ca = ctxTg.ap()
            wa = woT.ap()
            for half in range(2):
                nc.sync.dma_start(out=ctx_t[:, 4 * half : 4 * half + 4, :], in_=bass.AP(
                    tensor=ca.tensor, offset=128 * R * 4 * half,
                    ap=[[R, 128], [128 * R, 4], [1, R]]))
                nc.sync.dma_start(out=wo_t[:, 4 * half : 4 * half + 4, :], in_=bass.AP(
                    tensor=wa.tensor, offset=128 * D * 4 * half,
                    ap=[[D, 128], [128 * D, 4], [1, D]]))
            rec_t = smp.tile([H, R], BF16)
            with nc.allow_low_precision(reason="softmax denom recip in bf16; residual dominates"):
                nc.vector.reciprocal(out=rec_t[:], in_=dnm_t[:])

            gb = smp.tile([128, D], F32)
            bb = smp.tile([128, D], F32)
            g_ap = gamma.ap()
            b_ap = beta.ap()
            nc.sync.dma_start(
                out=gb[:], in_=bass.AP(tensor=g_ap.tensor, offset=g_ap.offset,
                                       ap=[[0, 128], [1, D]])
            )
            nc.sync.dma_start(
                out=bb[:], in_=bass.AP(tensor=b_ap.tensor, offset=b_ap.offset,
                                       ap=[[0, 128], [1, D]])
            )
            eps_t = smp.tile([128, 1], F32)
            nc.vector.memset(eps_t[:], EPS)

            # normalize ctx^T by per-(head, row) denominators -> bf16 tiles,
            # pipelined into the out-projection accumulation
            ctxn = cxp.tile([128, NIC, R], BF16)
            pos = {}
            for sc in range(4):
                po_t = psp.tile([128, D], F32, tag=f"po{sc % 2}", name=f"po_{sc}")
                pos[sc] = po_t
            for ic in range(NIC):
                pb = pbp.tile([128, R], F32, tag="pb")
                nc.tensor.matmul(pb[:], e16[:, ic, :], rec_t[:], start=True, stop=True)
                nc.vector.tensor_tensor(
                    out=ctxn[:, ic, :], in0=pb[:], in1=ctx_t[:, ic, :],
                    op=mybir.AluOpType.mult,
                )
                for sc in range(2):
                    for j in range(2):
                        nc.tensor.matmul(
                            pos[sc][:, 512 * j : 512 * j + 512],
                            ctxn[:, ic, 128 * sc : 128 * sc + 128],
                            wo_t[:, ic, 512 * j : 512 * j + 512],
                            start=(ic == 0),
                            stop=(ic == NIC - 1),
                        )

            for sc in range(4):
                po = pos[sc]
                if sc >= 2:
                    for j in range(2):
                        for ic in range(NIC):
                            nc.tensor.matmul(
                                po[:, 512 * j : 512 * j + 512],
                                ctxn[:, ic, 128 * sc : 128 * sc + 128],
                                wo_t[:, ic, 512 * j : 512 * j + 512],
                                start=(ic == 0),
                                stop=(ic == NIC - 1),
                            )
                xq_sb = wkp.tile([128, D], F32, tag="xq")
                nc.sync.dma_start(out=xq_sb[:], in_=xres[128 * sc : 128 * sc + 128, :])
                x_sb = wkp.tile([128, D], F32, tag="x")
                nc.vector.tensor_add(out=x_sb[:], in0=po[:], in1=xq_sb[:])

                stats = wkp.tile([128, 2, 6], F32, tag="bn")
                for g in range(2):
                    nc.vector.bn_stats(out=stats[:, g, :], in_=x_sb[:, 512 * g : 512 * g + 512])
                mv = wkp.tile([128, 2], F32, tag="mv")
                nc.vector.bn_aggr(out=mv[:], in_=stats[:])
                std = wkp.tile([128, 1], F32, tag="std")
                nc.scalar.activation(
                    out=std[:], in_=mv[:, 1:2],
                    func=mybir.ActivationFunctionType.Sqrt,
                    bias=eps_t[:], scale=1.0,
                )
                rstd = wkp.tile([128, 1], F32, tag="rstd")
                nc.vector.reciprocal(out=rstd[:], in_=std[:])
                xn = wkp.tile([128, D], F32, tag="xn")
                nc.vector.tensor_scalar(
                    out=xn[:], in0=x_sb[:],
                    scalar1=mv[:, 0:1], scalar2=rstd[:],
                    op0=mybir.AluOpType.subtract, op1=mybir.AluOpType.mult,
                )
                xg = wkp.tile([128, D], F32, tag="xg")
                nc.vector.tensor_mul(out=xg[:], in0=xn[:], in1=gb[:])
                xb = wkp.tile([128, D], F32, tag="xb")
                nc.vector.tensor_add(out=xb[:], in0=xg[:], in1=bb[:])
                nc.sync.dma_start(out=out[128 * sc : 128 * sc + 128, :], in_=xb[:])

    nc.compile()
    return nc


def _get(name):
    if name not in _cache:
        _cache[name] = build_kernel1() if name == "k1" else build_kernel2()
    return _cache[name]


def kernel(query, key, value, w_q, w_k, w_v, w_o, ln_gamma, ln_beta):
    query = np.asarray(query, np.float32)
    key = np.asarray(key, np.float32)
    value = np.asarray(value, np.float32)
    w_q = np.asarray(w_q, np.float32)
    w_k = np.asarray(w_k, np.float32)
    w_v = np.asarray(w_v, np.float32)
    w_o = np.asarray(w_o, np.float32)
    ln_gamma = np.asarray(ln_gamma, np.float32)
    ln_beta = np.asarray(ln_beta, np.float32)

    nc1 = _get("k1")
    nc2 = _get("k2")

    xqT = [np.ascontiguousarray(query[b].T).astype(NPFP8) for b in range(B)]
    xkT = [np.ascontiguousarray(key[b].T).astype(NPFP8) for b in range(B)]
    xvT = [np.ascontiguousarray(value[b].T).astype(NPFP8) for b in range(B)]
    # 16x scale keeps Xavier weights in e4m3 normal range (exact 2^k)
    wqT = np.ascontiguousarray(w_q.T * 16.0).astype(NPFP8)
    wkT = np.ascontiguousarray(w_k.T * 16.0).astype(NPFP8)
    wvT = np.ascontiguousarray(w_v.T * 16.0).astype(NPFP8)

    in_maps1 = []
    for c in range(N_CORES):
        b, hg = c // 4, c % 4
        in_maps1.append({
            "xq": xqT[b], "xk": xkT[b], "xv": xvT[b],
            "wq": np.ascontiguousarray(wqT[:, 256 * hg : 256 * hg + 256]),
            "wk": np.ascontiguousarray(wkT[:, 256 * hg : 256 * hg + 256]),
            "wv": np.ascontiguousarray(wvT[:, 256 * hg : 256 * hg + 256]),
        })
    res1 = run_bass_kernel_spmd(nc1, in_maps1, core_ids=list(range(N_CORES)))

    ctx_full = np.empty((D, B * S), NPBF16)
    dnm_full = np.empty((H, B * S), NPBF16)
    for c in range(N_CORES):
        b, hg = c // 4, c % 4
        cu = res1.results[c]["ctxu"]  # [65*4, 2048]
        for h in range(H_LOC):
            ctx_full[256 * hg + 64 * h : 256 * hg + 64 * h + 64, S * b : S * b + S] = \
                cu[65 * h : 65 * h + 64]
            dnm_full[4 * hg + h, S * b : S * b + S] = cu[65 * h + 64]

    woT = np.ascontiguousarray(w_o.T).astype(NPBF16)
    q_flat = query.reshape(B * S, D)
    g2 = ln_gamma.reshape(1, D)
    b2 = ln_beta.reshape(1, D)

    in_maps2 = []
    for c in range(N_CORES):
        r0 = 512 * c
        in_maps2.append({
            "ctxTg": np.ascontiguousarray(ctx_full[:, r0 : r0 + 512]),
            "dnm": np.ascontiguousarray(dnm_full[:, r0 : r0 + 512]),
            "e16": E16_HOST,
            "woT": woT,
            "xres": np.ascontiguousarray(q_flat[r0 : r0 + 512, :]),
            "gamma": g2, "beta": b2,
        })
    res2 = run_bass_kernel_spmd(nc2, in_maps2, core_ids=list(range(N_CORES)))

    out = np.concatenate([res2.results[c]["out"] for c in range(N_CORES)], axis=0)
    return out.reshape(B, S, D)


# revision 25
# speedup vs baseline: 1.0694x; 1.0282x over previous
"""Multi-head attention + residual + LayerNorm on 8 Trainium2 cores.

Sharding (per spec hint): core c = (batch b = c//4, head-group hg = c%4 of
4 heads).  Two SPMD launches:

Launch 1 (attention, per core):
  - Q/K/V projections as fp8e4m3 DoubleRow matmuls (2 k-tiles per pass,
    0.5 cycles/row).  The 1/8 score scale is folded into the Q weights;
    a +5.0625 score bias comes from a constant contraction row (2.25 in
    both Q and K operands), so PSUM scores arrive as s/8 + 5.0625.
  - scores^T per (head, k-chunk, q-block) as one DoubleRow matmul with
    d_k split 2x32 (dk-split layout built by an SBUF->SBUF DMA regroup
    after the projections).
  - softmax exp is split across TWO engines: ACT computes
    exp(in - 4.85577) -> fp8e4m3; DVE computes the same value (x1.2296,
    cancels in softmax) with an integer exp trick: u8 = trunc(max(
    in*11.5416, 0)) bit-cast as fp8e4m3.  Split tunable via SLOT_ENG.
  - ctx^T accumulated with fp8 DoubleRow matmuls over k-chunk pairs; a
    ones-column in the V operand makes row 64 the softmax denominator.
    ctx (65 rows: 64 dims + denom) leaves unnormalized in bf16.

Launch 2 (normalize + out-projection + residual + LayerNorm, 512 rows):
  reciprocal of the denominators, e16-matmul broadcast, one psum x sbuf
  multiply -> fp8 ctx, fp8 DoubleRow out-projection, residual add,
  bn_stats LayerNorm; gamma/beta applied on Pool to keep DVE short.
"""

from contextlib import ExitStack

import numpy as np
import ml_dtypes

import concourse.bass as bass
import concourse.bacc as bacc
import concourse.tile as tile
from concourse import mybir
from concourse.bass_utils import run_bass_kernel_spmd

BF16 = mybir.dt.bfloat16
F32 = mybir.dt.float32
FP8 = mybir.dt.float8e4
FP8E5 = mybir.dt.float8e5
U8 = mybir.dt.uint8
NPBF16 = ml_dtypes.bfloat16
NPFP8 = ml_dtypes.float8_e4m3
DR = mybir.MatmulPerfMode.DoubleRow

B, S, D = 2, 2048, 1024
H = 16
DK = 64
N_CORES = 8
H_LOC = 4           # heads per core
NCH = S // 128      # 16 k-chunks
NIC = D // 128      # 8 contraction chunks (4 DoubleRow pairs)
NQB = S // 512      # 4 q-blocks
EPS = 1e-5

# exp bias trick: scores arrive as s/8 + CBIAS (CBIAS = 2.25*2.25 via a
# constant contraction row).  DVE: u8 = trunc(max(in*EXPA, 0)) is the
# fp8e4m3 bit pattern of 1.2296*exp(s/8).  ACT matches via exp(in+ABIAS).
CBIAS = 8.0  # 2.0 * 4.0 const row; scores/8 in [-7.5, 9.5] all fit e5m2
EXPA = 5.7707801635558535  # 4*log2(e) for e5m2 bits
PSC = 1.0 / 2048.0   # psum scores carry 256x (16x-scaled Q,K); target score/8
ABIAS = -10.445207698461314  # -15*ln2 + ln(0.95313); matches DVE e5m2 trick

# exp slot -> engine map per (h, qb) block: 8 slots of 2 k-chunks.
# 'A' = ACT, 'D' = DVE (vector); two patterns alternate by block parity.
SLOT_PATS = ("ADADADAD", "AADADADA")

E16_HOST = np.zeros((H, NIC * 128), NPBF16)
for _ic in range(NIC):
    for _j in range(2):
        E16_HOST[2 * _ic + _j, 128 * _ic + 64 * _j : 128 * _ic + 64 * _j + 64] = 1.0

_cache = {}


def build_kernel1():
    nc = bacc.Bacc("TRN2", target_bir_lowering=False, debug=False)

    xq = nc.dram_tensor("xq", [D, S], FP8, kind="ExternalInput")
    xk = nc.dram_tensor("xk", [D, S], FP8, kind="ExternalInput")
    xv = nc.dram_tensor("xv", [D, S], FP8, kind="ExternalInput")
    wq = nc.dram_tensor("wq", [D, 256], FP8, kind="ExternalInput")
    wk = nc.dram_tensor("wk", [D, 256], FP8, kind="ExternalInput")
    wv = nc.dram_tensor("wv", [D, 256], FP8, kind="ExternalInput")
    cst = nc.dram_tensor("cst", [4, 512], FP8, kind="ExternalInput")  # rows: 128, 0, 16, abias-f32(bitcast)
    # 65 rows per head: 64 ctx dims + softmax denominator (unnormalized)
    ctxu = nc.dram_tensor("ctxu", [65 * H_LOC, S], BF16, kind="ExternalOutput")

    with tile.TileContext(nc) as tc:
        with (
            tc.tile_pool(name="wp", bufs=1) as wp,
            tc.tile_pool(name="xp", bufs=1) as xp,
            tc.tile_pool(name="qks", bufs=1) as qksp,
            tc.tile_pool(name="va", bufs=1) as vap,
            tc.tile_pool(name="sx", bufs=3) as sxp,
            tc.tile_pool(name="cu", bufs=2) as cup,
            tc.tile_pool(name="sm", bufs=1) as smp,
            tc.tile_pool(name="ps", bufs=3, space="PSUM") as psp,      # score slots [128,1024] x3
            tc.tile_pool(name="pc", bufs=2, space="PSUM") as pcp,      # ctx [96,512] x2
        ):
            w_q = wp.tile([128, NIC, 256], FP8)
            w_k = wp.tile([128, NIC, 256], FP8)
            w_v = wp.tile([128, NIC, 256], FP8)
            x_q = xp.tile([128, NQB, NIC, 512], FP8)
            x_k = xp.tile([128, NQB, NIC, 512], FP8)
            x_v = xp.tile([128, NQB, NIC, 512], FP8)
            # dk-split Q/K: [33, h, pair, S]; row 32 pair0 = 2.25 (bias row)
            qs = qksp.tile([33, H_LOC, NQB, 2, 512], FP8)
            ks = qksp.tile([33, H_LOC, NQB, 2, 512], FP8)
            # staging for projection output (natural head-dim layout)
            qstg = qksp.tile([128, 2, 2, S], FP8)  # [dims, (q|k), hp, S]
            vaug = vap.tile([128, NCH, H_LOC, 96], FP8)
            abias = smp.tile([128, 1], F32)

            cap = cst.ap()
            def cbcast(out_ap, row, dims):
                # broadcast const row over all free dims (innermost real)
                inner = dims[-1]
                ap = [[0, d] for d in dims[:-1]] + [[1, inner]]
                nc.sync.dma_start(out=out_ap, in_=bass.AP(
                    tensor=cap.tensor, offset=512 * row, ap=ap))
            cbcast(abias[:].bitcast(FP8), 3, [128, 4])
            cbcast(vaug[:, :, :, 64:96].rearrange("p a b c -> p (a b) c"), 2, [128, NCH * H_LOC, 32])
            cbcast(qs[32:33, :, :, 0, :].rearrange("p a b c -> p (a b) c"), 0, [1, H_LOC * NQB, 512])
            cbcast(qs[32:33, :, :, 1, :].rearrange("p a b c -> p (a b) c"), 1, [1, H_LOC * NQB, 512])
            cbcast(ks[32:33, :, :, 0, :].rearrange("p a b c -> p (a b) c"), 0, [1, H_LOC * NQB, 512])
            cbcast(ks[32:33, :, :, 1, :].rearrange("p a b c -> p (a b) c"), 1, [1, H_LOC * NQB, 512])

            # ---- input DMA: one multi-dim DMA per tensor ----
            def load_w(tile, dram):
                a = dram.ap()
                nc.sync.dma_start(out=tile[:], in_=bass.AP(
                    tensor=a.tensor, offset=0,
                    ap=[[256, 128], [128 * 256, NIC], [1, 256]]))

            def load_x(tile, dram, split=False):
                a = dram.ap()
                if split:
                    for qb in range(NQB):
                        nc.sync.dma_start(out=tile[:, qb, :, :], in_=bass.AP(
                            tensor=a.tensor, offset=512 * qb,
                            ap=[[S, 128], [128 * S, NIC], [1, 512]]))
                else:
                    nc.sync.dma_start(out=tile[:], in_=bass.AP(
                        tensor=a.tensor, offset=0,
                        ap=[[S, 128], [512, NQB], [128 * S, NIC], [1, 512]]))

            load_w(w_q, wq)
            load_w(w_k, wk)
            load_x(x_q, xq, split=True)
            load_x(x_k, xk, split=True)
            load_w(w_v, wv)
            load_x(x_v, xv)

            # ---- Q/K projections (fp8 DoubleRow), hp-granular ----
            def qk_proj(t, hp):
                xt = (x_q, x_k)[t]
                wt = (w_q, w_k)[t]
                for qb in range(NQB):
                    pt = psp.tile([128, 1024], F32, tag="slot")
                    for icp in range(NIC // 2):
                        nc.tensor.matmul(
                            pt[:, 0:512],
                            wt[:, 2 * icp : 2 * icp + 2, 128 * hp : 128 * hp + 128],
                            xt[:, qb, 2 * icp : 2 * icp + 2, :],
                            start=(icp == 0),
                            stop=(icp == NIC // 2 - 1),
                            perf_mode=DR,
                        )
                    nc.scalar.copy(
                        out=qstg[:, t, hp, 512 * qb : 512 * qb + 512], in_=pt[:, 0:512]
                    )

            def qk_regroup(t, hp):
                # [128, S] staging -> dk-split [32, h, j, S] (+bias row set above)
                dst = (qs, ks)[t]
                for blk in range(4):
                    h, j = 2 * hp + blk // 2, blk % 2
                    nc.sync.dma_start(
                        out=dst[0:32, h, :, j, :],
                        in_=qstg[32 * blk : 32 * blk + 32, t, hp, :],
                    )

            def v_proj(c):
                pvt = psp.tile([128, 1024], F32, tag="slot")
                for icp in range(NIC // 2):
                    nc.tensor.matmul(
                        pvt[:, 0:256],
                        x_v[:, c // 4, 2 * icp : 2 * icp + 2, 128 * (c % 4) : 128 * (c % 4) + 128],
                        w_v[:, 2 * icp : 2 * icp + 2, :],
                        start=(icp == 0),
                        stop=(icp == NIC // 2 - 1),
                        perf_mode=DR,
                    )
                nc.vector.tensor_copy(out=vaug[:, c, :, 0:64], in_=pvt[:, 0:256])

            for hp in range(2):
                qk_proj(0, hp)
                qk_proj(1, hp)
                qk_regroup(0, hp)
                qk_regroup(1, hp)

            # ---- main attention loop ----
            blocks = [(h, qb) for h in range(H_LOC) for qb in range(NQB)]
            exps = {}    # (h, qb) -> expS tile
            pctxs = {}   # (h, qb) -> ctx psum

            def scores_block(h, qb, vblk=None):
                ex = sxp.tile([128, NCH, 512], FP8E5, tag="expS")
                exps[(h, qb)] = ex
                for sl in range(8):
                    if vblk is not None:
                        v_proj(8 * vblk + sl)
                    slot = psp.tile([128, 1024], F32, tag="slot")
                    for k in range(2):
                        c = 2 * sl + k
                        nc.tensor.matmul(
                            slot[:, 512 * k : 512 * k + 512],
                            ks[0:33, h, c // 4, :, 128 * (c % 4) : 128 * (c % 4) + 128],
                            qs[0:33, h, qb, :, :],
                            start=True,
                            stop=True,
                            perf_mode=DR,
                        )
                    eng = SLOT_PATS[(H_LOC * h + qb) % 2][sl]
                    dst = ex[:, 2 * sl : 2 * sl + 2, :]
                    if eng == "A":
                        nc.scalar.activation(
                            out=dst,
                            in_=slot[:],
                            func=mybir.ActivationFunctionType.Exp,
                            bias=abias[:],
                            scale=PSC,
                        )
                    else:
                        nc.vector.tensor_scalar(
                            out=dst.bitcast(U8),
                            in0=slot[:],
                            scalar1=EXPA * PSC,
                            scalar2=0.0,
                            op0=mybir.AluOpType.mult,
                            op1=mybir.AluOpType.max,
                        )

            def ctx_block(h, qb):
                ex = exps.pop((h, qb))
                pctx = pcp.tile([96, 512], F32, tag="ctx")
                pctxs[(h, qb)] = pctx
                for i in range(8):
                    nc.tensor.matmul(
                        pctx[:],
                        vaug[:, 2 * i : 2 * i + 2, h, :],
                        ex[:, 2 * i : 2 * i + 2, :],
                        start=(i == 0),
                        stop=(i == 7),
                        perf_mode=DR,
                    )

            def finish_block(h, qb):
                pctx = pctxs.pop((h, qb))
                cu = cup.tile([65, 512], BF16, tag="cu")
                if (H_LOC * h + qb) % 2 == 0:
                    nc.scalar.copy(out=cu[:], in_=pctx[0:65, :])
                else:
                    nc.vector.tensor_copy(out=cu[:], in_=pctx[0:65, :])
                nc.sync.dma_start(
                    out=ctxu[65 * h : 65 * h + 65, 512 * qb : 512 * qb + 512],
                    in_=cu[:],
                )

            prev = None
            for bi, blk in enumerate(blocks):
                scores_block(*blk, vblk=bi if bi < 2 else None)
                if prev is not None:
                    ctx_block(*prev)
                    finish_block(*prev)
                prev = blk
            ctx_block(*prev)
            finish_block(*prev)

    nc.compile()
    return nc


def build_kernel2():
    nc = bacc.Bacc("TRN2", target_bir_lowering=False, debug=False)

    R = 512  # rows per core
    ctxTg = nc.dram_tensor("ctxTg", [D, R], BF16, kind="ExternalInput")
    dnm = nc.dram_tensor("dnm", [H, R], BF16, kind="ExternalInput")
    e16d = nc.dram_tensor("e16", [H, NIC * 128], BF16, kind="ExternalInput")
    woT = nc.dram_tensor("woT", [D, D], BF16, kind="ExternalInput")
    xres = nc.dram_tensor("xres", [R, D], F32, kind="ExternalInput")
    gamma = nc.dram_tensor("gamma", [1, D], F32, kind="ExternalInput")
    beta = nc.dram_tensor("beta", [1, D], F32, kind="ExternalInput")
    out = nc.dram_tensor("out", [R, D], F32, kind="ExternalOutput")

    with tile.TileContext(nc) as tc:
        with (
            tc.tile_pool(name="wo", bufs=1) as wop,
            tc.tile_pool(name="cx", bufs=1) as cxp,
            tc.tile_pool(name="sm", bufs=1) as smp,
            tc.tile_pool(name="wk", bufs=3) as wkp,
            tc.tile_pool(name="ps", bufs=1, space="PSUM") as psp,
            tc.tile_pool(name="pb", bufs=2, space="PSUM") as pbp,
        ):
            wo_t = wop.tile([128, NIC, D], BF16)
            ctx_t = cxp.tile([128, NIC, R], BF16)
            dnm_t = smp.tile([H, R], BF16)
            nc.sync.dma_start(out=dnm_t[:], in_=dnm[:])
            e16 = smp.tile([H, NIC, 128], BF16)
            nc.sync.dma_start(out=e16[:].rearrange("h a b -> h (a b)"), in_=e16d[:])
            ca = ctxTg.ap()
            wa = woT.ap()
            for half in range(2):
                nc.sync.dma_start(out=ctx_t[:, 4 * half : 4 * half + 4, :], in_=bass.AP(
                    tensor=ca.tensor, offset=128 * R * 4 * half,
                    ap=[[R, 128], [128 * R, 4], [1, R]]))
                nc.sync.dma_start(out=wo_t[:, 4 * half : 4 * half + 4, :], in_=bass.AP(
                    tensor=wa.tensor, offset=128 * D * 4 * half,
                    ap=[[D, 128], [128 * D, 4], [1, D]]))
            rec_t = smp.tile([H, R], BF16)
            with nc.allow_low_precision(reason="softmax denom recip in bf16; residual dominates"):
                nc.vector.reciprocal(out=rec_t[:], in_=dnm_t[:])

            gb = smp.tile([128, D], F32)
            bb = smp.tile([128, D], F32)
            g_ap = gamma.ap()
            b_ap = beta.ap()
            nc.sync.dma_start(
                out=gb[:], in_=bass.AP(tensor=g_ap.tensor, offset=g_ap.offset,
                                       ap=[[0, 128], [1, D]])
            )
            nc.sync.dma_start(
                out=bb[:], in_=bass.AP(tensor=b_ap.tensor, offset=b_ap.offset,
                                       ap=[[0, 128], [1, D]])
            )
            eps_t = smp.tile([128, 1], F32)
            nc.vector.memset(eps_t[:], EPS)

            # normalize ctx^T by per-(head, row) denominators -> bf16 tiles,
            # pipelined into the out-projection accumulation
            ctxn = cxp.tile([128, NIC, R], BF16)
            pos = {}
            for sc in range(4):
                po_t = psp.tile([128, D], F32, tag=f"po{sc % 2}", name=f"po_{sc}")
                pos[sc] = po_t
            for ic in range(NIC):
                pb = pbp.tile([128, R], F32, tag="pb")
                nc.tensor.matmul(pb[:], e16[:, ic, :], rec_t[:], start=True, stop=True)
                nc.vector.tensor_tensor(
                    out=ctxn[:, ic, :], in0=pb[:], in1=ctx_t[:, ic, :],
                    op=mybir.AluOpType.mult,
                )
                for sc in range(2):
                    for j in range(2):
                        nc.tensor.matmul(
                            pos[sc][:, 512 * j : 512 * j + 512],
                            ctxn[:, ic, 128 * sc : 128 * sc + 128],
                            wo_t[:, ic, 512 * j : 512 * j + 512],
                            start=(ic == 0),
                            stop=(ic == NIC - 1),
                        )

            for sc in range(4):
                po = pos[sc]
                if sc >= 2:
                    for j in range(2):
                        for ic in range(NIC):
                            nc.tensor.matmul(
                                po[:, 512 * j : 512 * j + 512],
                                ctxn[:, ic, 128 * sc : 128 * sc + 128],
                                wo_t[:, ic, 512 * j : 512 * j + 512],
                                start=(ic == 0),
                                stop=(ic == NIC - 1),
                            )
                xq_sb = wkp.tile([128, D], F32, tag="xq")
                nc.sync.dma_start(out=xq_sb[:], in_=xres[128 * sc : 128 * sc + 128, :])
                x_sb = wkp.tile([128, D], F32, tag="x")
                nc.vector.tensor_add(out=x_sb[:], in0=po[:], in1=xq_sb[:])

                stats = wkp.tile([128, 2, 6], F32, tag="bn")
                for g in range(2):
                    nc.vector.bn_stats(out=stats[:, g, :], in_=x_sb[:, 512 * g : 512 * g + 512])
                mv = wkp.tile([128, 2], F32, tag="mv")
                nc.vector.bn_aggr(out=mv[:], in_=stats[:])
                std = wkp.tile([128, 1], F32, tag="std")
                nc.scalar.activation(
                    out=std[:], in_=mv[:, 1:2],
                    func=mybir.ActivationFunctionType.Sqrt,
                    bias=eps_t[:], scale=1.0,
                )
                rstd = wkp.tile([128, 1], F32, tag="rstd")
                nc.vector.reciprocal(out=rstd[:], in_=std[:])
                xn = wkp.tile([128, D], F32, tag="xn")
                nc.vector.tensor_scalar(
                    out=xn[:], in0=x_sb[:],
                    scalar1=mv[:, 0:1], scalar2=rstd[:],
                    op0=mybir.AluOpType.subtract, op1=mybir.AluOpType.mult,
                )
                e = nc.gpsimd if sc < 2 else nc.vector
                xg = wkp.tile([128, D], F32, tag="xg")
                e.tensor_mul(out=xg[:], in0=xn[:], in1=gb[:])
                xb = wkp.tile([128, D], F32, tag="xb")
                e.tensor_add(out=xb[:], in0=xg[:], in1=bb[:])
                nc.sync.dma_start(out=out[128 * sc : 128 * sc + 128, :], in_=xb[:])

    nc.compile()
    return nc


def _get(name):
    if name not in _cache:
        _cache[name] = build_kernel1() if name == "k1" else build_kernel2()
    return _cache[name]


def kernel(query, key, value, w_q, w_k, w_v, w_o, ln_gamma, ln_beta):
    query = np.asarray(query, np.float32)
    key = np.asarray(key, np.float32)
    value = np.asarray(value, np.float32)
    w_q = np.asarray(w_q, np.float32)
    w_k = np.asarray(w_k, np.float32)
    w_v = np.asarray(w_v, np.float32)
    w_o = np.asarray(w_o, np.float32)
    ln_gamma = np.asarray(ln_gamma, np.float32)
    ln_beta = np.asarray(ln_beta, np.float32)

    nc1 = _get("k1")
    nc2 = _get("k2")

    xqT = [np.ascontiguousarray(query[b].T).astype(NPFP8) for b in range(B)]
    xkT = [np.ascontiguousarray(key[b].T).astype(NPFP8) for b in range(B)]
    xvT = [np.ascontiguousarray(value[b].T).astype(NPFP8) for b in range(B)]
    # 16x scale keeps Xavier weights in e4m3 normal range (exact 2^k)
    wqT = np.ascontiguousarray(w_q.T * 16.0).astype(NPFP8)
    wkT = np.ascontiguousarray(w_k.T * 16.0).astype(NPFP8)
    wvT = np.ascontiguousarray(w_v.T * 16.0).astype(NPFP8)

    cst = np.zeros((4, 512), NPFP8)
    cst[0] = 128.0
    cst[1] = 0.0
    cst[2] = 16.0
    # row 3 carries ABIAS as raw f32 bytes (kernel bitcasts back to f32)
    cst[3] = np.frombuffer(np.full(128, ABIAS, np.float32).tobytes(), dtype=NPFP8)
    in_maps1 = []
    for c in range(N_CORES):
        b, hg = c // 4, c % 4
        in_maps1.append({
            "xq": xqT[b], "xk": xkT[b], "xv": xvT[b], "cst": cst,
            "wq": np.ascontiguousarray(wqT[:, 256 * hg : 256 * hg + 256]),
            "wk": np.ascontiguousarray(wkT[:, 256 * hg : 256 * hg + 256]),
            "wv": np.ascontiguousarray(wvT[:, 256 * hg : 256 * hg + 256]),
        })
    res1 = run_bass_kernel_spmd(nc1, in_maps1, core_ids=list(range(N_CORES)))

    ctx_full = np.empty((D, B * S), NPBF16)
    dnm_full = np.empty((H, B * S), NPBF16)
    for c in range(N_CORES):
        b, hg = c // 4, c % 4
        cu = res1.results[c]["ctxu"]  # [65*4, 2048]
        for h in range(H_LOC):
            ctx_full[256 * hg + 64 * h : 256 * hg + 64 * h + 64, S * b : S * b + S] = \
                cu[65 * h : 65 * h + 64]
            dnm_full[4 * hg + h, S * b : S * b + S] = cu[65 * h + 64]

    woT = np.ascontiguousarray(w_o.T).astype(NPBF16)
    q_flat = query.reshape(B * S, D)
    g2 = ln_gamma.reshape(1, D)
    b2 = ln_beta.reshape(1, D)

    in_maps2 = []
    for c in range(N_CORES):
        r0 = 512 * c
        in_maps2.append({
            "ctxTg": np.ascontiguousarray(ctx_full[:, r0 : r0 + 512]),
            "dnm": np.ascontiguousarray(dnm_full[:, r0 : r0 + 512]),
            "e16": E16_HOST,
            "woT": woT,
            "xres": np.ascontiguousarray(q_flat[r0 : r0 + 512, :]),
            "gamma": g2, "beta": b2,
        })
    res2 = run_bass_kernel_spmd(nc2, in_maps2, core_ids=list(range(N_CORES)))

    out = np.concatenate([res2.results[c]["out"] for c in range(N_CORES)], axis=0)
    return out.reshape(B, S, D)


# revision 26
# speedup vs baseline: 1.0795x; 1.0095x over previous
"""Multi-head attention + residual + LayerNorm on 8 Trainium2 cores.

Sharding (per spec hint): core c = (batch b = c//4, head-group hg = c%4 of
4 heads).  Two SPMD launches:

Launch 1 (attention, per core):
  - Q/K/V projections as fp8e4m3 DoubleRow matmuls (2 k-tiles per pass,
    0.5 cycles/row).  The 1/8 score scale is folded into the Q weights;
    a +5.0625 score bias comes from a constant contraction row (2.25 in
    both Q and K operands), so PSUM scores arrive as s/8 + 5.0625.
  - scores^T per (head, k-chunk, q-block) as one DoubleRow matmul with
    d_k split 2x32 (dk-split layout built by an SBUF->SBUF DMA regroup
    after the projections).
  - softmax exp is split across TWO engines: ACT computes
    exp(in - 4.85577) -> fp8e4m3; DVE computes the same value (x1.2296,
    cancels in softmax) with an integer exp trick: u8 = trunc(max(
    in*11.5416, 0)) bit-cast as fp8e4m3.  Split tunable via SLOT_ENG.
  - ctx^T accumulated with fp8 DoubleRow matmuls over k-chunk pairs; a
    ones-column in the V operand makes row 64 the softmax denominator.
    ctx (65 rows: 64 dims + denom) leaves unnormalized in bf16.

Launch 2 (normalize + out-projection + residual + LayerNorm, 512 rows):
  reciprocal of the denominators, e16-matmul broadcast, one psum x sbuf
  multiply -> fp8 ctx, fp8 DoubleRow out-projection, residual add,
  bn_stats LayerNorm; gamma/beta applied on Pool to keep DVE short.
"""

from contextlib import ExitStack

import numpy as np
import ml_dtypes

import concourse.bass as bass
import concourse.bacc as bacc
import concourse.tile as tile
from concourse import mybir
from concourse.bass_utils import run_bass_kernel_spmd

BF16 = mybir.dt.bfloat16
F32 = mybir.dt.float32
FP8 = mybir.dt.float8e4
FP8E5 = mybir.dt.float8e5
U8 = mybir.dt.uint8
NPBF16 = ml_dtypes.bfloat16
NPFP8 = ml_dtypes.float8_e4m3
DR = mybir.MatmulPerfMode.DoubleRow

B, S, D = 2, 2048, 1024
H = 16
DK = 64
N_CORES = 8
H_LOC = 4           # heads per core
NCH = S // 128      # 16 k-chunks
NIC = D // 128      # 8 contraction chunks (4 DoubleRow pairs)
NQB = S // 512      # 4 q-blocks
EPS = 1e-5

# exp bias trick: scores arrive as s/8 + CBIAS (CBIAS = 2.25*2.25 via a
# constant contraction row).  DVE: u8 = trunc(max(in*EXPA, 0)) is the
# fp8e4m3 bit pattern of 1.2296*exp(s/8).  ACT matches via exp(in+ABIAS).
CBIAS = 8.0  # 2.0 * 4.0 const row; scores/8 in [-7.5, 9.5] all fit e5m2
EXPA = 5.7707801635558535  # 4*log2(e) for e5m2 bits
PSC = 1.0 / 2048.0   # psum scores carry 256x (16x-scaled Q,K); target score/8
ABIAS = -10.445207698461314  # -15*ln2 + ln(0.95313); matches DVE e5m2 trick

# exp slot -> engine map per (h, qb) block: 8 slots of 2 k-chunks.
# 'A' = ACT, 'D' = DVE (vector); two patterns alternate by block parity.
SLOT_PATS = ("ADADADAD", "AADADADA")

E16_HOST = np.zeros((H, NIC * 128), NPBF16)
for _ic in range(NIC):
    for _j in range(2):
        E16_HOST[2 * _ic + _j, 128 * _ic + 64 * _j : 128 * _ic + 64 * _j + 64] = 1.0

_cache = {}


def build_kernel1():
    nc = bacc.Bacc("TRN2", target_bir_lowering=False, debug=False)

    xq = nc.dram_tensor("xq", [D, S], FP8, kind="ExternalInput")
    xk = nc.dram_tensor("xk", [D, S], FP8, kind="ExternalInput")
    xv = nc.dram_tensor("xv", [D, S], FP8, kind="ExternalInput")
    wq = nc.dram_tensor("wq", [D, 256], FP8, kind="ExternalInput")
    wk = nc.dram_tensor("wk", [D, 256], FP8, kind="ExternalInput")
    wv = nc.dram_tensor("wv", [D, 256], FP8, kind="ExternalInput")
    cst = nc.dram_tensor("cst", [4, 512], FP8, kind="ExternalInput")  # rows: 128, 0, 16, abias-f32(bitcast)
    # 65 rows per head: 64 ctx dims + softmax denominator (unnormalized)
    ctxu = nc.dram_tensor("ctxu", [65 * H_LOC, S], BF16, kind="ExternalOutput")

    with tile.TileContext(nc) as tc:
        with (
            tc.tile_pool(name="wp", bufs=1) as wp,
            tc.tile_pool(name="xp", bufs=1) as xp,
            tc.tile_pool(name="qks", bufs=1) as qksp,
            tc.tile_pool(name="va", bufs=1) as vap,
            tc.tile_pool(name="sx", bufs=3) as sxp,
            tc.tile_pool(name="cu", bufs=2) as cup,
            tc.tile_pool(name="sm", bufs=1) as smp,
            tc.tile_pool(name="ps", bufs=3, space="PSUM") as psp,      # score slots [128,1024] x3
            tc.tile_pool(name="pc", bufs=2, space="PSUM") as pcp,      # ctx [96,512] x2
        ):
            w_q = wp.tile([128, NIC, 256], FP8)
            w_k = wp.tile([128, NIC, 256], FP8)
            w_v = wp.tile([128, NIC, 256], FP8)
            x_q = xp.tile([128, NQB, NIC, 512], FP8)
            x_k = xp.tile([128, NQB, NIC, 512], FP8)
            x_v = xp.tile([128, NQB, NIC, 512], FP8)
            # dk-split Q/K: [33, h, pair, S]; row 32 pair0 = 2.25 (bias row)
            qs = qksp.tile([33, H_LOC, NQB, 2, 512], FP8)
            ks = qksp.tile([33, H_LOC, NQB, 2, 512], FP8)
            # staging for projection output (natural head-dim layout)
            qstg = qksp.tile([128, 2, 2, S], FP8)  # [dims, (q|k), hp, S]
            vaug = vap.tile([128, NCH, H_LOC, 96], FP8)
            abias = smp.tile([128, 1], F32)

            cap = cst.ap()
            def cbcast(out_ap, row, dims):
                # broadcast const row over all free dims (innermost real)
                inner = dims[-1]
                ap = [[0, d] for d in dims[:-1]] + [[1, inner]]
                nc.sync.dma_start(out=out_ap, in_=bass.AP(
                    tensor=cap.tensor, offset=512 * row, ap=ap))
            cbcast(abias[:].bitcast(FP8), 3, [128, 4])
            cbcast(vaug[:, :, :, 64:96].rearrange("p a b c -> p (a b) c"), 2, [128, NCH * H_LOC, 32])
            cbcast(qs[32:33, :, :, 0, :].rearrange("p a b c -> p (a b) c"), 0, [1, H_LOC * NQB, 512])
            cbcast(qs[32:33, :, :, 1, :].rearrange("p a b c -> p (a b) c"), 1, [1, H_LOC * NQB, 512])
            cbcast(ks[32:33, :, :, 0, :].rearrange("p a b c -> p (a b) c"), 0, [1, H_LOC * NQB, 512])
            cbcast(ks[32:33, :, :, 1, :].rearrange("p a b c -> p (a b) c"), 1, [1, H_LOC * NQB, 512])

            # ---- input DMA: one multi-dim DMA per tensor ----
            def load_w(tile, dram):
                a = dram.ap()
                nc.sync.dma_start(out=tile[:], in_=bass.AP(
                    tensor=a.tensor, offset=0,
                    ap=[[256, 128], [128 * 256, NIC], [1, 256]]))

            def load_x(tile, dram, split=False):
                a = dram.ap()
                if split:
                    for qb in range(NQB):
                        nc.sync.dma_start(out=tile[:, qb, :, :], in_=bass.AP(
                            tensor=a.tensor, offset=512 * qb,
                            ap=[[S, 128], [128 * S, NIC], [1, 512]]))
                else:
                    nc.sync.dma_start(out=tile[:], in_=bass.AP(
                        tensor=a.tensor, offset=0,
                        ap=[[S, 128], [512, NQB], [128 * S, NIC], [1, 512]]))

            load_w(w_q, wq)
            load_w(w_k, wk)
            load_x(x_q, xq, split=True)
            load_x(x_k, xk, split=True)
            load_w(w_v, wv)
            load_x(x_v, xv)

            # ---- Q/K projections (fp8 DoubleRow), hp-granular ----
            def qk_proj(t, hp):
                xt = (x_q, x_k)[t]
                wt = (w_q, w_k)[t]
                for qb in range(NQB):
                    pt = psp.tile([128, 1024], F32, tag="slot")
                    for icp in range(NIC // 2):
                        nc.tensor.matmul(
                            pt[:, 0:512],
                            wt[:, 2 * icp : 2 * icp + 2, 128 * hp : 128 * hp + 128],
                            xt[:, qb, 2 * icp : 2 * icp + 2, :],
                            start=(icp == 0),
                            stop=(icp == NIC // 2 - 1),
                            perf_mode=DR,
                        )
                    nc.scalar.copy(
                        out=qstg[:, t, hp, 512 * qb : 512 * qb + 512], in_=pt[:, 0:512]
                    )

            def qk_regroup(t, hp):
                # [128, S] staging -> dk-split [32, h, j, S] (+bias row set above)
                dst = (qs, ks)[t]
                for blk in range(4):
                    h, j = 2 * hp + blk // 2, blk % 2
                    nc.sync.dma_start(
                        out=dst[0:32, h, :, j, :],
                        in_=qstg[32 * blk : 32 * blk + 32, t, hp, :],
                    )

            def v_proj(c):
                pvt = pcp.tile([128, 512], F32, tag="ctx")
                for icp in range(NIC // 2):
                    nc.tensor.matmul(
                        pvt[:, 0:256],
                        x_v[:, c // 4, 2 * icp : 2 * icp + 2, 128 * (c % 4) : 128 * (c % 4) + 128],
                        w_v[:, 2 * icp : 2 * icp + 2, :],
                        start=(icp == 0),
                        stop=(icp == NIC // 2 - 1),
                        perf_mode=DR,
                    )
                nc.vector.tensor_copy(out=vaug[:, c, :, 0:64], in_=pvt[:, 0:256])

            for hp in range(2):
                qk_proj(0, hp)
                qk_proj(1, hp)
                qk_regroup(0, hp)
                qk_regroup(1, hp)

            # ---- main attention loop ----
            blocks = [(h, qb) for h in range(H_LOC) for qb in range(NQB)]
            exps = {}    # (h, qb) -> expS tile
            pctxs = {}   # (h, qb) -> ctx psum

            def scores_block(h, qb, vblk=None):
                ex = sxp.tile([128, NCH, 512], FP8E5, tag="expS")
                exps[(h, qb)] = ex
                for sl in range(8):
                    if vblk is not None:
                        v_proj(8 * vblk + sl)
                    slot = psp.tile([128, 1024], F32, tag="slot")
                    for k in range(2):
                        c = 2 * sl + k
                        nc.tensor.matmul(
                            slot[:, 512 * k : 512 * k + 512],
                            ks[0:33, h, c // 4, :, 128 * (c % 4) : 128 * (c % 4) + 128],
                            qs[0:33, h, qb, :, :],
                            start=True,
                            stop=True,
                            perf_mode=DR,
                        )
                    eng = SLOT_PATS[(H_LOC * h + qb) % 2][sl]
                    dst = ex[:, 2 * sl : 2 * sl + 2, :]
                    if eng == "A":
                        nc.scalar.activation(
                            out=dst,
                            in_=slot[:],
                            func=mybir.ActivationFunctionType.Exp,
                            bias=abias[:],
                            scale=PSC,
                        )
                    else:
                        nc.vector.tensor_scalar(
                            out=dst.bitcast(U8),
                            in0=slot[:],
                            scalar1=EXPA * PSC,
                            scalar2=0.0,
                            op0=mybir.AluOpType.mult,
                            op1=mybir.AluOpType.max,
                        )

            def ctx_block(h, qb):
                ex = exps.pop((h, qb))
                pctx = pcp.tile([96, 512], F32, tag="ctx")
                pctxs[(h, qb)] = pctx
                for i in range(8):
                    nc.tensor.matmul(
                        pctx[:],
                        vaug[:, 2 * i : 2 * i + 2, h, :],
                        ex[:, 2 * i : 2 * i + 2, :],
                        start=(i == 0),
                        stop=(i == 7),
                        perf_mode=DR,
                    )

            def finish_block(h, qb):
                pctx = pctxs.pop((h, qb))
                cu = cup.tile([65, 512], BF16, tag="cu")
                if (H_LOC * h + qb) % 2 == 0:
                    nc.scalar.copy(out=cu[:], in_=pctx[0:65, :])
                else:
                    nc.vector.tensor_copy(out=cu[:], in_=pctx[0:65, :])
                nc.sync.dma_start(
                    out=ctxu[65 * h : 65 * h + 65, 512 * qb : 512 * qb + 512],
                    in_=cu[:],
                )

            prev = None
            for bi, blk in enumerate(blocks):
                scores_block(*blk, vblk=bi if bi < 2 else None)
                if prev is not None:
                    ctx_block(*prev)
                    finish_block(*prev)
                prev = blk
            ctx_block(*prev)
            finish_block(*prev)

    nc.compile()
    return nc


def build_kernel2():
    nc = bacc.Bacc("TRN2", target_bir_lowering=False, debug=False)

    R = 512  # rows per core
    ctxTg = nc.dram_tensor("ctxTg", [D, R], BF16, kind="ExternalInput")
    dnm = nc.dram_tensor("dnm", [H, R], BF16, kind="ExternalInput")
    e16d = nc.dram_tensor("e16", [H, NIC * 128], BF16, kind="ExternalInput")
    woT = nc.dram_tensor("woT", [D, D], BF16, kind="ExternalInput")
    xres = nc.dram_tensor("xres", [R, D], F32, kind="ExternalInput")
    gamma = nc.dram_tensor("gamma", [1, D], F32, kind="ExternalInput")
    beta = nc.dram_tensor("beta", [1, D], F32, kind="ExternalInput")
    out = nc.dram_tensor("out", [R, D], F32, kind="ExternalOutput")

    with tile.TileContext(nc) as tc:
        with (
            tc.tile_pool(name="wo", bufs=1) as wop,
            tc.tile_pool(name="cx", bufs=1) as cxp,
            tc.tile_pool(name="sm", bufs=1) as smp,
            tc.tile_pool(name="wk", bufs=3) as wkp,
            tc.tile_pool(name="ps", bufs=1, space="PSUM") as psp,
            tc.tile_pool(name="pb", bufs=2, space="PSUM") as pbp,
        ):
            wo_t = wop.tile([128, NIC, D], BF16)
            ctx_t = cxp.tile([128, NIC, R], BF16)
            dnm_t = smp.tile([H, R], BF16)
            nc.sync.dma_start(out=dnm_t[:], in_=dnm[:])
            e16 = smp.tile([H, NIC, 128], BF16)
            nc.sync.dma_start(out=e16[:].rearrange("h a b -> h (a b)"), in_=e16d[:])
            ca = ctxTg.ap()
            wa = woT.ap()
            for half in range(2):
                nc.sync.dma_start(out=ctx_t[:, 4 * half : 4 * half + 4, :], in_=bass.AP(
                    tensor=ca.tensor, offset=128 * R * 4 * half,
                    ap=[[R, 128], [128 * R, 4], [1, R]]))
                nc.sync.dma_start(out=wo_t[:, 4 * half : 4 * half + 4, :], in_=bass.AP(
                    tensor=wa.tensor, offset=128 * D * 4 * half,
                    ap=[[D, 128], [128 * D, 4], [1, D]]))
            rec_t = smp.tile([H, R], BF16)
            with nc.allow_low_precision(reason="softmax denom recip in bf16; residual dominates"):
                nc.vector.reciprocal(out=rec_t[:], in_=dnm_t[:])

            gb = smp.tile([128, D], F32)
            bb = smp.tile([128, D], F32)
            g_ap = gamma.ap()
            b_ap = beta.ap()
            nc.sync.dma_start(
                out=gb[:], in_=bass.AP(tensor=g_ap.tensor, offset=g_ap.offset,
                                       ap=[[0, 128], [1, D]])
            )
            nc.sync.dma_start(
                out=bb[:], in_=bass.AP(tensor=b_ap.tensor, offset=b_ap.offset,
                                       ap=[[0, 128], [1, D]])
            )
            eps_t = smp.tile([128, 1], F32)
            nc.vector.memset(eps_t[:], EPS)

            # normalize ctx^T by per-(head, row) denominators -> bf16 tiles,
            # pipelined into the out-projection accumulation
            ctxn = cxp.tile([128, NIC, R], BF16)
            pos = {}
            for sc in range(4):
                po_t = psp.tile([128, D], F32, tag=f"po{sc % 2}", name=f"po_{sc}")
                pos[sc] = po_t
            for ic in range(NIC):
                pb = pbp.tile([128, R], F32, tag="pb")
                nc.tensor.matmul(pb[:], e16[:, ic, :], rec_t[:], start=True, stop=True)
                nc.vector.tensor_tensor(
                    out=ctxn[:, ic, :], in0=pb[:], in1=ctx_t[:, ic, :],
                    op=mybir.AluOpType.mult,
                )
                for sc in range(2):
                    for j in range(2):
                        nc.tensor.matmul(
                            pos[sc][:, 512 * j : 512 * j + 512],
                            ctxn[:, ic, 128 * sc : 128 * sc + 128],
                            wo_t[:, ic, 512 * j : 512 * j + 512],
                            start=(ic == 0),
                            stop=(ic == NIC - 1),
                        )

            for sc in range(4):
                po = pos[sc]
                if sc >= 2:
                    for j in range(2):
                        for ic in range(NIC):
                            nc.tensor.matmul(
                                po[:, 512 * j : 512 * j + 512],
                                ctxn[:, ic, 128 * sc : 128 * sc + 128],
                                wo_t[:, ic, 512 * j : 512 * j + 512],
                                start=(ic == 0),
                                stop=(ic == NIC - 1),
                            )
                xq_sb = wkp.tile([128, D], F32, tag="xq")
                nc.sync.dma_start(out=xq_sb[:], in_=xres[128 * sc : 128 * sc + 128, :])
                x_sb = wkp.tile([128, D], F32, tag="x")
                nc.vector.tensor_add(out=x_sb[:], in0=po[:], in1=xq_sb[:])

                stats = wkp.tile([128, 2, 6], F32, tag="bn")
                for g in range(2):
                    nc.vector.bn_stats(out=stats[:, g, :], in_=x_sb[:, 512 * g : 512 * g + 512])
                mv = wkp.tile([128, 2], F32, tag="mv")
                nc.vector.bn_aggr(out=mv[:], in_=stats[:])
                std = wkp.tile([128, 1], F32, tag="std")
                nc.scalar.activation(
                    out=std[:], in_=mv[:, 1:2],
                    func=mybir.ActivationFunctionType.Sqrt,
                    bias=eps_t[:], scale=1.0,
                )
                rstd = wkp.tile([128, 1], F32, tag="rstd")
                nc.vector.reciprocal(out=rstd[:], in_=std[:])
                xn = wkp.tile([128, D], F32, tag="xn")
                nc.vector.tensor_scalar(
                    out=xn[:], in0=x_sb[:],
                    scalar1=mv[:, 0:1], scalar2=rstd[:],
                    op0=mybir.AluOpType.subtract, op1=mybir.AluOpType.mult,
                )
                e = nc.gpsimd if sc < 2 else nc.vector
                xg = wkp.tile([128, D], F32, tag="xg")
                e.tensor_mul(out=xg[:], in0=xn[:], in1=gb[:])
                xb = wkp.tile([128, D], F32, tag="xb")
                e.tensor_add(out=xb[:], in0=xg[:], in1=bb[:])
                nc.sync.dma_start(out=out[128 * sc : 128 * sc + 128, :], in_=xb[:])

    nc.compile()
    return nc


def _get(name):
    if name not in _cache:
        _cache[name] = build_kernel1() if name == "k1" else build_kernel2()
    return _cache[name]


def kernel(query, key, value, w_q, w_k, w_v, w_o, ln_gamma, ln_beta):
    query = np.asarray(query, np.float32)
    key = np.asarray(key, np.float32)
    value = np.asarray(value, np.float32)
    w_q = np.asarray(w_q, np.float32)
    w_k = np.asarray(w_k, np.float32)
    w_v = np.asarray(w_v, np.float32)
    w_o = np.asarray(w_o, np.float32)
    ln_gamma = np.asarray(ln_gamma, np.float32)
    ln_beta = np.asarray(ln_beta, np.float32)

    nc1 = _get("k1")
    nc2 = _get("k2")

    xqT = [np.ascontiguousarray(query[b].T).astype(NPFP8) for b in range(B)]
    xkT = [np.ascontiguousarray(key[b].T).astype(NPFP8) for b in range(B)]
    xvT = [np.ascontiguousarray(value[b].T).astype(NPFP8) for b in range(B)]
    # 16x scale keeps Xavier weights in e4m3 normal range (exact 2^k)
    wqT = np.ascontiguousarray(w_q.T * 16.0).astype(NPFP8)
    wkT = np.ascontiguousarray(w_k.T * 16.0).astype(NPFP8)
    wvT = np.ascontiguousarray(w_v.T * 16.0).astype(NPFP8)

    cst = np.zeros((4, 512), NPFP8)
    cst[0] = 128.0
    cst[1] = 0.0
    cst[2] = 16.0
    # row 3 carries ABIAS as raw f32 bytes (kernel bitcasts back to f32)
    cst[3] = np.frombuffer(np.full(128, ABIAS, np.float32).tobytes(), dtype=NPFP8)
    in_maps1 = []
    for c in range(N_CORES):
        b, hg = c // 4, c % 4
        in_maps1.append({
            "xq": xqT[b], "xk": xkT[b], "xv": xvT[b], "cst": cst,
            "wq": np.ascontiguousarray(wqT[:, 256 * hg : 256 * hg + 256]),
            "wk": np.ascontiguousarray(wkT[:, 256 * hg : 256 * hg + 256]),
            "wv": np.ascontiguousarray(wvT[:, 256 * hg : 256 * hg + 256]),
        })
    res1 = run_bass_kernel_spmd(nc1, in_maps1, core_ids=list(range(N_CORES)))

    ctx_full = np.empty((D, B * S), NPBF16)
    dnm_full = np.empty((H, B * S), NPBF16)
    for c in range(N_CORES):
        b, hg = c // 4, c % 4
        cu = res1.results[c]["ctxu"]  # [65*4, 2048]
        for h in range(H_LOC):
            ctx_full[256 * hg + 64 * h : 256 * hg + 64 * h + 64, S * b : S * b + S] = \
                cu[65 * h : 65 * h + 64]
            dnm_full[4 * hg + h, S * b : S * b + S] = cu[65 * h + 64]

    woT = np.ascontiguousarray(w_o.T).astype(NPBF16)
    q_flat = query.reshape(B * S, D)
    g2 = ln_gamma.reshape(1, D)
    b2 = ln_beta.reshape(1, D)

    in_maps2 = []
    for c in range(N_CORES):
        r0 = 512 * c
        in_maps2.append({
            "ctxTg": np.ascontiguousarray(ctx_full[:, r0 : r0 + 512]),
            "dnm": np.ascontiguousarray(dnm_full[:, r0 : r0 + 512]),
            "e16": E16_HOST,
            "woT": woT,
            "xres": np.ascontiguousarray(q_flat[r0 : r0 + 512, :]),
            "gamma": g2, "beta": b2,
        })
    res2 = run_bass_kernel_spmd(nc2, in_maps2, core_ids=list(range(N_CORES)))

    out = np.concatenate([res2.results[c]["out"] for c in range(N_CORES)], axis=0)
    return out.reshape(B, S, D)


# revision 27
# speedup vs baseline: 1.1038x; 1.0225x over previous
"""Multi-head attention + residual + LayerNorm on 8 Trainium2 cores.

Sharding (per spec hint): core c = (batch b = c//4, head-group hg = c%4 of
4 heads).  Two SPMD launches:

Launch 1 (attention, per core):
  - Q/K/V projections as fp8e4m3 DoubleRow matmuls (2 k-tiles per pass,
    0.5 cycles/row).  The 1/8 score scale is folded into the Q weights;
    a +5.0625 score bias comes from a constant contraction row (2.25 in
    both Q and K operands), so PSUM scores arrive as s/8 + 5.0625.
  - scores^T per (head, k-chunk, q-block) as one DoubleRow matmul with
    d_k split 2x32 (dk-split layout built by an SBUF->SBUF DMA regroup
    after the projections).
  - softmax exp is split across TWO engines: ACT computes
    exp(in - 4.85577) -> fp8e4m3; DVE computes the same value (x1.2296,
    cancels in softmax) with an integer exp trick: u8 = trunc(max(
    in*11.5416, 0)) bit-cast as fp8e4m3.  Split tunable via SLOT_ENG.
  - ctx^T accumulated with fp8 DoubleRow matmuls over k-chunk pairs; a
    ones-column in the V operand makes row 64 the softmax denominator.
    ctx (65 rows: 64 dims + denom) leaves unnormalized in bf16.

Launch 2 (normalize + out-projection + residual + LayerNorm, 512 rows):
  reciprocal of the denominators, e16-matmul broadcast, one psum x sbuf
  multiply -> fp8 ctx, fp8 DoubleRow out-projection, residual add,
  bn_stats LayerNorm; gamma/beta applied on Pool to keep DVE short.
"""

from contextlib import ExitStack

import numpy as np
import ml_dtypes

import concourse.bass as bass
import concourse.bacc as bacc
import concourse.tile as tile
from concourse import mybir
from concourse.bass_utils import run_bass_kernel_spmd

BF16 = mybir.dt.bfloat16
F32 = mybir.dt.float32
FP8 = mybir.dt.float8e4
FP8E5 = mybir.dt.float8e5
U8 = mybir.dt.uint8
NPBF16 = ml_dtypes.bfloat16
NPFP8 = ml_dtypes.float8_e4m3
DR = mybir.MatmulPerfMode.DoubleRow

B, S, D = 2, 2048, 1024
H = 16
DK = 64
N_CORES = 8
H_LOC = 4           # heads per core
NCH = S // 128      # 16 k-chunks
NIC = D // 128      # 8 contraction chunks (4 DoubleRow pairs)
NQB = S // 512      # 4 q-blocks
EPS = 1e-5

# exp bias trick: scores arrive as s/8 + CBIAS (CBIAS = 2.25*2.25 via a
# constant contraction row).  DVE: u8 = trunc(max(in*EXPA, 0)) is the
# fp8e4m3 bit pattern of 1.2296*exp(s/8).  ACT matches via exp(in+ABIAS).
CBIAS = 8.0  # 2.0 * 4.0 const row; scores/8 in [-7.5, 9.5] all fit e5m2
EXPA = 5.7707801635558535  # 4*log2(e) for e5m2 bits
PSC = 1.0 / 2048.0   # psum scores carry 256x (16x-scaled Q,K); target score/8
ABIAS = -10.445207698461314  # -15*ln2 + ln(0.95313); matches DVE e5m2 trick

# exp slot -> engine map per (h, qb) block: 8 slots of 2 k-chunks.
# 'A' = ACT, 'D' = DVE (vector); two patterns alternate by block parity.
SLOT_PATS = ("ADADADAD", "AADADADA")

E16_HOST = np.zeros((H, NIC * 128), NPBF16)
for _ic in range(NIC):
    for _j in range(2):
        E16_HOST[2 * _ic + _j, 128 * _ic + 64 * _j : 128 * _ic + 64 * _j + 64] = 1.0

_cache = {}


def build_kernel1():
    nc = bacc.Bacc("TRN2", target_bir_lowering=False, debug=False)

    xq = nc.dram_tensor("xq", [D, S], FP8, kind="ExternalInput")
    xk = nc.dram_tensor("xk", [D, S], FP8, kind="ExternalInput")
    xv = nc.dram_tensor("xv", [D, S], FP8, kind="ExternalInput")
    wq = nc.dram_tensor("wq", [D, 256], FP8, kind="ExternalInput")
    wk = nc.dram_tensor("wk", [D, 256], FP8, kind="ExternalInput")
    wv = nc.dram_tensor("wv", [D, 256], FP8, kind="ExternalInput")
    cst = nc.dram_tensor("cst", [4, 512], FP8, kind="ExternalInput")  # rows: 128, 0, 16, abias-f32(bitcast)
    # 65 rows per head: 64 ctx dims + softmax denominator (unnormalized)
    ctxu = nc.dram_tensor("ctxu", [65 * H_LOC, S], BF16, kind="ExternalOutput")

    with tile.TileContext(nc) as tc:
        with (
            tc.tile_pool(name="wp", bufs=1) as wp,
            tc.tile_pool(name="xp", bufs=1) as xp,
            tc.tile_pool(name="qks", bufs=1) as qksp,
            tc.tile_pool(name="va", bufs=1) as vap,
            tc.tile_pool(name="sx", bufs=3) as sxp,
            tc.tile_pool(name="cu", bufs=2) as cup,
            tc.tile_pool(name="sm", bufs=1) as smp,
            tc.tile_pool(name="ps", bufs=3, space="PSUM") as psp,      # score slots [128,1024] x3
            tc.tile_pool(name="pc", bufs=2, space="PSUM") as pcp,      # ctx [96,512] x2
        ):
            w_q = wp.tile([128, NIC, 256], FP8)
            w_k = wp.tile([128, NIC, 256], FP8)
            w_v = wp.tile([128, NIC, 256], FP8)
            x_q = xp.tile([128, NQB, NIC, 512], FP8)
            x_k = xp.tile([128, NQB, NIC, 512], FP8)
            x_v = xp.tile([128, NQB, NIC, 512], FP8)
            # dk-split Q/K: [33, h, pair, S]; row 32 pair0 = 2.25 (bias row)
            qs = qksp.tile([33, H_LOC, NQB, 2, 512], FP8)
            ks = qksp.tile([33, H_LOC, NQB, 2, 512], FP8)
            # staging for projection output (natural head-dim layout)
            qstg = qksp.tile([128, 2, 2, S], FP8)  # [dims, (q|k), hp, S]
            vaug = vap.tile([128, NCH, H_LOC, 96], FP8)
            abias = smp.tile([128, 1], F32)

            cap = cst.ap()
            def cbcast(out_ap, row, dims):
                # broadcast const row over all free dims (innermost real)
                inner = dims[-1]
                ap = [[0, d] for d in dims[:-1]] + [[1, inner]]
                nc.sync.dma_start(out=out_ap, in_=bass.AP(
                    tensor=cap.tensor, offset=512 * row, ap=ap))
            cbcast(abias[:].bitcast(FP8), 3, [128, 4])
            cbcast(vaug[:, :, :, 64:96].rearrange("p a b c -> p (a b) c"), 2, [128, NCH * H_LOC, 32])
            cbcast(qs[32:33, :, :, 0, :].rearrange("p a b c -> p (a b) c"), 0, [1, H_LOC * NQB, 512])
            cbcast(qs[32:33, :, :, 1, :].rearrange("p a b c -> p (a b) c"), 1, [1, H_LOC * NQB, 512])
            cbcast(ks[32:33, :, :, 0, :].rearrange("p a b c -> p (a b) c"), 0, [1, H_LOC * NQB, 512])
            cbcast(ks[32:33, :, :, 1, :].rearrange("p a b c -> p (a b) c"), 1, [1, H_LOC * NQB, 512])

            # ---- input DMA: one multi-dim DMA per tensor ----
            def load_w(tile, dram):
                a = dram.ap()
                nc.sync.dma_start(out=tile[:], in_=bass.AP(
                    tensor=a.tensor, offset=0,
                    ap=[[256, 128], [128 * 256, NIC], [1, 256]]))

            def load_x(tile, dram, split=False):
                a = dram.ap()
                if split:
                    for qb in range(NQB):
                        nc.sync.dma_start(out=tile[:, qb, :, :], in_=bass.AP(
                            tensor=a.tensor, offset=512 * qb,
                            ap=[[S, 128], [128 * S, NIC], [1, 512]]))
                else:
                    nc.sync.dma_start(out=tile[:], in_=bass.AP(
                        tensor=a.tensor, offset=0,
                        ap=[[S, 128], [512, NQB], [128 * S, NIC], [1, 512]]))

            load_w(w_q, wq)
            load_w(w_k, wk)
            load_x(x_q, xq, split=True)
            load_x(x_k, xk, split=True)
            load_w(w_v, wv)
            load_x(x_v, xv)

            # ---- Q/K projections (fp8 DoubleRow), hp-granular ----
            def qk_proj(t, hp):
                xt = (x_q, x_k)[t]
                wt = (w_q, w_k)[t]
                for qb in range(NQB):
                    pt = psp.tile([128, 1024], F32, tag="slot")
                    for icp in range(NIC // 2):
                        nc.tensor.matmul(
                            pt[:, 0:512],
                            wt[:, 2 * icp : 2 * icp + 2, 128 * hp : 128 * hp + 128],
                            xt[:, qb, 2 * icp : 2 * icp + 2, :],
                            start=(icp == 0),
                            stop=(icp == NIC // 2 - 1),
                            perf_mode=DR,
                        )
                    nc.scalar.copy(
                        out=qstg[:, t, hp, 512 * qb : 512 * qb + 512], in_=pt[:, 0:512]
                    )

            def qk_regroup(t, hp):
                # [128, S] staging -> dk-split [32, h, j, S] (+bias row set above)
                dst = (qs, ks)[t]
                for blk in range(4):
                    h, j = 2 * hp + blk // 2, blk % 2
                    nc.sync.dma_start(
                        out=dst[0:32, h, :, j, :],
                        in_=qstg[32 * blk : 32 * blk + 32, t, hp, :],
                    )

            def v_proj(c):
                pvt = pcp.tile([128, 512], F32, tag="ctx")
                for icp in range(NIC // 2):
                    nc.tensor.matmul(
                        pvt[:, 0:256],
                        x_v[:, c // 4, 2 * icp : 2 * icp + 2, 128 * (c % 4) : 128 * (c % 4) + 128],
                        w_v[:, 2 * icp : 2 * icp + 2, :],
                        start=(icp == 0),
                        stop=(icp == NIC // 2 - 1),
                        perf_mode=DR,
                    )
                nc.vector.tensor_copy(out=vaug[:, c, :, 0:64], in_=pvt[:, 0:256])

            for hp in range(2):
                qk_proj(0, hp)
                qk_proj(1, hp)
                qk_regroup(0, hp)
                qk_regroup(1, hp)

            # ---- main attention loop ----
            blocks = [(h, qb) for h in range(H_LOC) for qb in range(NQB)]
            exps = {}    # (h, qb) -> expS tile
            pctxs = {}   # (h, qb) -> ctx psum

            def scores_block(h, qb, vblk=None):
                ex = sxp.tile([128, NCH, 512], FP8E5, tag="expS")
                exps[(h, qb)] = ex
                for sl in range(8):
                    if vblk is not None:
                        v_proj(8 * vblk + sl)
                    slot = psp.tile([128, 1024], F32, tag="slot")
                    for k in range(2):
                        c = 2 * sl + k
                        nc.tensor.matmul(
                            slot[:, 512 * k : 512 * k + 512],
                            ks[0:33, h, c // 4, :, 128 * (c % 4) : 128 * (c % 4) + 128],
                            qs[0:33, h, qb, :, :],
                            start=True,
                            stop=True,
                            perf_mode=DR,
                        )
                    eng = SLOT_PATS[(H_LOC * h + qb) % 2][sl]
                    dst = ex[:, 2 * sl : 2 * sl + 2, :]
                    if eng == "A":
                        nc.scalar.activation(
                            out=dst,
                            in_=slot[:],
                            func=mybir.ActivationFunctionType.Exp,
                            bias=abias[:],
                            scale=PSC,
                        )
                    else:
                        nc.vector.tensor_scalar(
                            out=dst.bitcast(U8),
                            in0=slot[:],
                            scalar1=EXPA * PSC,
                            scalar2=0.0,
                            op0=mybir.AluOpType.mult,
                            op1=mybir.AluOpType.max,
                        )

            def ctx_block(h, qb):
                ex = exps.pop((h, qb))
                pctx = pcp.tile([96, 512], F32, tag="ctx")
                pctxs[(h, qb)] = pctx
                for i in range(8):
                    nc.tensor.matmul(
                        pctx[:],
                        vaug[:, 2 * i : 2 * i + 2, h, :],
                        ex[:, 2 * i : 2 * i + 2, :],
                        start=(i == 0),
                        stop=(i == 7),
                        perf_mode=DR,
                    )

            def finish_block(h, qb):
                pctx = pctxs.pop((h, qb))
                cu = cup.tile([65, 512], BF16, tag="cu")
                nc.vector.tensor_copy(out=cu[:], in_=pctx[0:65, :])
                nc.sync.dma_start(
                    out=ctxu[65 * h : 65 * h + 65, 512 * qb : 512 * qb + 512],
                    in_=cu[:],
                )

            prev = None
            for bi, blk in enumerate(blocks):
                scores_block(*blk, vblk=bi if bi < 2 else None)
                if prev is not None:
                    ctx_block(*prev)
                    finish_block(*prev)
                prev = blk
            ctx_block(*prev)
            finish_block(*prev)

    nc.compile()
    return nc


def build_kernel2():
    nc = bacc.Bacc("TRN2", target_bir_lowering=False, debug=False)

    R = 512  # rows per core
    ctxTg = nc.dram_tensor("ctxTg", [D, R], BF16, kind="ExternalInput")
    dnm = nc.dram_tensor("dnm", [H, R], BF16, kind="ExternalInput")
    e16d = nc.dram_tensor("e16", [H, NIC * 128], BF16, kind="ExternalInput")
    woT = nc.dram_tensor("woT", [D, D], BF16, kind="ExternalInput")
    xres = nc.dram_tensor("xres", [R, D], F32, kind="ExternalInput")
    gamma = nc.dram_tensor("gamma", [1, D], F32, kind="ExternalInput")
    beta = nc.dram_tensor("beta", [1, D], F32, kind="ExternalInput")
    out = nc.dram_tensor("out", [R, D], F32, kind="ExternalOutput")

    with tile.TileContext(nc) as tc:
        with (
            tc.tile_pool(name="wo", bufs=1) as wop,
            tc.tile_pool(name="cx", bufs=1) as cxp,
            tc.tile_pool(name="sm", bufs=1) as smp,
            tc.tile_pool(name="wk", bufs=3) as wkp,
            tc.tile_pool(name="ps", bufs=1, space="PSUM") as psp,
            tc.tile_pool(name="pb", bufs=2, space="PSUM") as pbp,
        ):
            wo_t = wop.tile([128, NIC, D], BF16)
            ctx_t = cxp.tile([128, NIC, R], BF16)
            dnm_t = smp.tile([H, R], BF16)
            nc.sync.dma_start(out=dnm_t[:], in_=dnm[:])
            e16 = smp.tile([H, NIC, 128], BF16)
            nc.sync.dma_start(out=e16[:].rearrange("h a b -> h (a b)"), in_=e16d[:])
            ca = ctxTg.ap()
            wa = woT.ap()
            for half in range(2):
                nc.sync.dma_start(out=ctx_t[:, 4 * half : 4 * half + 4, :], in_=bass.AP(
                    tensor=ca.tensor, offset=128 * R * 4 * half,
                    ap=[[R, 128], [128 * R, 4], [1, R]]))
                nc.sync.dma_start(out=wo_t[:, 4 * half : 4 * half + 4, :], in_=bass.AP(
                    tensor=wa.tensor, offset=128 * D * 4 * half,
                    ap=[[D, 128], [128 * D, 4], [1, D]]))
            rec_t = smp.tile([H, R], BF16)
            with nc.allow_low_precision(reason="softmax denom recip in bf16; residual dominates"):
                nc.vector.reciprocal(out=rec_t[:], in_=dnm_t[:])

            gb = smp.tile([128, D], F32)
            bb = smp.tile([128, D], F32)
            g_ap = gamma.ap()
            b_ap = beta.ap()
            nc.sync.dma_start(
                out=gb[:], in_=bass.AP(tensor=g_ap.tensor, offset=g_ap.offset,
                                       ap=[[0, 128], [1, D]])
            )
            nc.sync.dma_start(
                out=bb[:], in_=bass.AP(tensor=b_ap.tensor, offset=b_ap.offset,
                                       ap=[[0, 128], [1, D]])
            )
            eps_t = smp.tile([128, 1], F32)
            nc.vector.memset(eps_t[:], EPS)

            # normalize ctx^T by per-(head, row) denominators -> bf16 tiles,
            # pipelined into the out-projection accumulation
            ctxn = cxp.tile([128, NIC, R], BF16)
            pos = {}
            for sc in range(4):
                po_t = psp.tile([128, D], F32, tag=f"po{sc % 2}", name=f"po_{sc}")
                pos[sc] = po_t
            for ic in range(NIC):
                pb = pbp.tile([128, R], F32, tag="pb")
                nc.tensor.matmul(pb[:], e16[:, ic, :], rec_t[:], start=True, stop=True)
                nc.vector.tensor_tensor(
                    out=ctxn[:, ic, :], in0=pb[:], in1=ctx_t[:, ic, :],
                    op=mybir.AluOpType.mult,
                )
                for sc in range(2):
                    for j in range(2):
                        nc.tensor.matmul(
                            pos[sc][:, 512 * j : 512 * j + 512],
                            ctxn[:, ic, 128 * sc : 128 * sc + 128],
                            wo_t[:, ic, 512 * j : 512 * j + 512],
                            start=(ic == 0),
                            stop=(ic == NIC - 1),
                        )

            for sc in range(4):
                po = pos[sc]
                if sc >= 2:
                    for j in range(2):
                        for ic in range(NIC):
                            nc.tensor.matmul(
                                po[:, 512 * j : 512 * j + 512],
                                ctxn[:, ic, 128 * sc : 128 * sc + 128],
                                wo_t[:, ic, 512 * j : 512 * j + 512],
                                start=(ic == 0),
                                stop=(ic == NIC - 1),
                            )
                xq_sb = wkp.tile([128, D], F32, tag="xq")
                nc.sync.dma_start(out=xq_sb[:], in_=xres[128 * sc : 128 * sc + 128, :])
                x_sb = wkp.tile([128, D], F32, tag="x")
                nc.vector.tensor_add(out=x_sb[:], in0=po[:], in1=xq_sb[:])

                stats = wkp.tile([128, 2, 6], F32, tag="bn")
                for g in range(2):
                    nc.vector.bn_stats(out=stats[:, g, :], in_=x_sb[:, 512 * g : 512 * g + 512])
                mv = wkp.tile([128, 2], F32, tag="mv")
                nc.vector.bn_aggr(out=mv[:], in_=stats[:])
                std = wkp.tile([128, 1], F32, tag="std")
                nc.scalar.activation(
                    out=std[:], in_=mv[:, 1:2],
                    func=mybir.ActivationFunctionType.Sqrt,
                    bias=eps_t[:], scale=1.0,
                )
                rstd = wkp.tile([128, 1], F32, tag="rstd")
                nc.vector.reciprocal(out=rstd[:], in_=std[:])
                xn = wkp.tile([128, D], F32, tag="xn")
                nc.vector.tensor_scalar(
                    out=xn[:], in0=x_sb[:],
                    scalar1=mv[:, 0:1], scalar2=rstd[:],
                    op0=mybir.AluOpType.subtract, op1=mybir.AluOpType.mult,
                )
                e = nc.gpsimd if sc < 2 else nc.vector
                xg = wkp.tile([128, D], F32, tag="xg")
                e.tensor_mul(out=xg[:], in0=xn[:], in1=gb[:])
                xb = wkp.tile([128, D], F32, tag="xb")
                e.tensor_add(out=xb[:], in0=xg[:], in1=bb[:])
                nc.sync.dma_start(out=out[128 * sc : 128 * sc + 128, :], in_=xb[:])

    nc.compile()
    return nc


def _get(name):
    if name not in _cache:
        _cache[name] = build_kernel1() if name == "k1" else build_kernel2()
    return _cache[name]


def kernel(query, key, value, w_q, w_k, w_v, w_o, ln_gamma, ln_beta):
    query = np.asarray(query, np.float32)
    key = np.asarray(key, np.float32)
    value = np.asarray(value, np.float32)
    w_q = np.asarray(w_q, np.float32)
    w_k = np.asarray(w_k, np.float32)
    w_v = np.asarray(w_v, np.float32)
    w_o = np.asarray(w_o, np.float32)
    ln_gamma = np.asarray(ln_gamma, np.float32)
    ln_beta = np.asarray(ln_beta, np.float32)

    nc1 = _get("k1")
    nc2 = _get("k2")

    xqT = [np.ascontiguousarray(query[b].T).astype(NPFP8) for b in range(B)]
    xkT = [np.ascontiguousarray(key[b].T).astype(NPFP8) for b in range(B)]
    xvT = [np.ascontiguousarray(value[b].T).astype(NPFP8) for b in range(B)]
    # 16x scale keeps Xavier weights in e4m3 normal range (exact 2^k)
    wqT = np.ascontiguousarray(w_q.T * 16.0).astype(NPFP8)
    wkT = np.ascontiguousarray(w_k.T * 16.0).astype(NPFP8)
    wvT = np.ascontiguousarray(w_v.T * 16.0).astype(NPFP8)

    cst = np.zeros((4, 512), NPFP8)
    cst[0] = 128.0
    cst[1] = 0.0
    cst[2] = 16.0
    # row 3 carries ABIAS as raw f32 bytes (kernel bitcasts back to f32)
    cst[3] = np.frombuffer(np.full(128, ABIAS, np.float32).tobytes(), dtype=NPFP8)
    in_maps1 = []
    for c in range(N_CORES):
        b, hg = c // 4, c % 4
        in_maps1.append({
            "xq": xqT[b], "xk": xkT[b], "xv": xvT[b], "cst": cst,
            "wq": np.ascontiguousarray(wqT[:, 256 * hg : 256 * hg + 256]),
            "wk": np.ascontiguousarray(wkT[:, 256 * hg : 256 * hg + 256]),
            "wv": np.ascontiguousarray(wvT[:, 256 * hg : 256 * hg + 256]),
        })
    res1 = run_bass_kernel_spmd(nc1, in_maps1, core_ids=list(range(N_CORES)))

    ctx_full = np.empty((D, B * S), NPBF16)
    dnm_full = np.empty((H, B * S), NPBF16)
    for c in range(N_CORES):
        b, hg = c // 4, c % 4
        cu = res1.results[c]["ctxu"]  # [65*4, 2048]
        for h in range(H_LOC):
            ctx_full[256 * hg + 64 * h : 256 * hg + 64 * h + 64, S * b : S * b + S] = \
                cu[65 * h : 65 * h + 64]
            dnm_full[4 * hg + h, S * b : S * b + S] = cu[65 * h + 64]

    woT = np.ascontiguousarray(w_o.T).astype(NPBF16)
    q_flat = query.reshape(B * S, D)
    g2 = ln_gamma.reshape(1, D)
    b2 = ln_beta.reshape(1, D)

    in_maps2 = []
    for c in range(N_CORES):
        r0 = 512 * c
        in_maps2.append({
            "ctxTg": np.ascontiguousarray(ctx_full[:, r0 : r0 + 512]),
            "dnm": np.ascontiguousarray(dnm_full[:, r0 : r0 + 512]),
            "e16": E16_HOST,
            "woT": woT,
            "xres": np.ascontiguousarray(q_flat[r0 : r0 + 512, :]),
            "gamma": g2, "beta": b2,
        })
    res2 = run_bass_kernel_spmd(nc2, in_maps2, core_ids=list(range(N_CORES)))

    out = np.concatenate([res2.results[c]["out"] for c in range(N_CORES)], axis=0)
    return out.reshape(B, S, D)
